# revision 1
# baseline (speedup 1.0000x reference)
"""DeepHamCritic (3x GCNConv + dense head) on 8 trn2 NeuronCores.

Strategy:
  - Host: build dense normalized adjacency A [1000,1000] from edge_index
    (self-loops + deg^-1/2 symmetric norm), pad nodes 1000 -> 1024.
  - GCN layers sharded by destination node (128 padded nodes per core):
    aggregation = dense matmul against the local A^T column slice,
    AllGather of node features between layers.
  - Dense head: Wd1 [512000,256] row-sharded (64Ki rows/core, fp16),
    streamed through SBUF slabs into a PE matvec accumulated in PSUM;
    partial [256] results AllGather'ed and summed on-chip; the tiny
    Wd2/Wd3/Wd4 layers are computed replicated on every core.
"""

import numpy as np

N_CORES = 8
N = 1000          # real nodes
P = 1024          # padded nodes
S = P // N_CORES  # nodes per core = 128
F = 128           # input features
D = 512           # GCN hidden
H = 256           # dense hidden
KCH = P * D // N_CORES // 128   # 512 f-chunks of 128 per core
SLAB_CH = 32                    # chunks per DMA slab
N_SLAB = KCH // SLAB_CH         # 16 slabs
SLAB_W = SLAB_CH * H            # 8192 fp16 cols per slab
WSLAB_BUFS = 7

_NC = None


def _build_nc(reps=1, mode="full"):
    global N_SLAB, SLAB_W
    N_SLAB = KCH // SLAB_CH
    SLAB_W = SLAB_CH * H
    import concourse.bacc as bacc
    import concourse.mybir as mybir
    import concourse.tile as tile

    f32 = mybir.dt.float32
    f16 = mybir.dt.float16
    RG = [list(range(N_CORES))]

    nc = bacc.Bacc("TRN2", target_bir_lowering=False, debug=False,
                   num_devices=N_CORES)

    # ---- I/O ----
    xk = nc.dram_tensor("xk", [P, F], f32, kind="ExternalInput")
    ats = nc.dram_tensor("ats", [P, S], f32, kind="ExternalInput")
    w1 = nc.dram_tensor("w1", [F, D], f32, kind="ExternalInput")
    w2 = nc.dram_tensor("w2", [D, D], f32, kind="ExternalInput")
    w3 = nc.dram_tensor("w3", [D, D], f32, kind="ExternalInput")
    b1bc = nc.dram_tensor("b1bc", [128, D], f32, kind="ExternalInput")
    b2bc = nc.dram_tensor("b2bc", [128, D], f32, kind="ExternalInput")
    b3bc = nc.dram_tensor("b3bc", [128, D], f32, kind="ExternalInput")
    ident = nc.dram_tensor("ident", [128, 128], f32, kind="ExternalInput")
    wd1s = nc.dram_tensor("wd1s", [128, KCH * H], f16, kind="ExternalInput")
    wd2 = nc.dram_tensor("wd2", [H, H], f32, kind="ExternalInput")
    wd3 = nc.dram_tensor("wd3", [H, H], f32, kind="ExternalInput")
    wd4 = nc.dram_tensor("wd4", [H, 1], f32, kind="ExternalInput")
    bd1c = nc.dram_tensor("bd1c", [128, 2], f32, kind="ExternalInput")
    bd2c = nc.dram_tensor("bd2c", [128, 2], f32, kind="ExternalInput")
    bd3c = nc.dram_tensor("bd3c", [128, 2], f32, kind="ExternalInput")
    bd4 = nc.dram_tensor("bd4", [1, 1], f32, kind="ExternalInput")
    out = nc.dram_tensor("out", [1, 1], f32, kind="ExternalOutput")

    Tanh = mybir.ActivationFunctionType.Tanh
    Bypass = mybir.AluOpType.bypass

    with tile.TileContext(nc) as tc:
        with (
            tc.tile_pool(name="wslab", bufs=WSLAB_BUFS) as wpool,
            tc.tile_pool(name="const", bufs=1) as cp,
            tc.tile_pool(name="hfull", bufs=8) as hp,
            tc.tile_pool(name="work", bufs=2) as wk,
            tc.tile_pool(name="psum", bufs=2, space="PSUM") as pp,
            tc.tile_pool(name="psacc", bufs=1, space="PSUM") as pacc,
            tc.tile_pool(name="dram", bufs=1, space="DRAM") as dp,
        ):
          static_slabs = None
          if mode == "head_pe":
            static_slabs = []
            for sidx in range(8):
                st = cp.tile([128, SLAB_W], f16, tag=f"sslab{sidx}")
                nc.vector.memset(st[:], 0.001)
                static_slabs.append(st)
          for _rep in range(reps):
            # ---- load constants ----
            xk_t = []
            ats_t = []
            for q in range(8):
                t = cp.tile([128, F], f32, tag=f"xk{q}")
                nc.sync.dma_start(t[:], xk[q * 128:(q + 1) * 128, :])
                xk_t.append(t)
                a = cp.tile([128, S], f32, tag=f"ats{q}")
                nc.sync.dma_start(a[:], ats[q * 128:(q + 1) * 128, :])
                ats_t.append(a)
            w1_t = cp.tile([F, D], f32, tag="w1")
            nc.sync.dma_start(w1_t[:], w1[:])
            w2_t = []
            w3_t = []
            for m in range(4):
                t2 = cp.tile([128, D], f32, tag=f"w2{m}")
                nc.sync.dma_start(t2[:], w2[m * 128:(m + 1) * 128, :])
                w2_t.append(t2)
                t3 = cp.tile([128, D], f32, tag=f"w3{m}")
                nc.sync.dma_start(t3[:], w3[m * 128:(m + 1) * 128, :])
                w3_t.append(t3)
            b_t = []
            for nm, hndl in (("b1", b1bc), ("b2", b2bc), ("b3", b3bc)):
                t = cp.tile([128, D], f32, tag=nm)
                nc.sync.dma_start(t[:], hndl[:])
                b_t.append(t)
            id_t = cp.tile([128, 128], f32, tag="ident")
            nc.sync.dma_start(id_t[:], ident[:])
            wd2_t = []
            wd3_t = []
            for k in range(2):
                t2 = cp.tile([128, H], f32, tag=f"wd2{k}")
                nc.sync.dma_start(t2[:], wd2[k * 128:(k + 1) * 128, :])
                wd2_t.append(t2)
                t3 = cp.tile([128, H], f32, tag=f"wd3{k}")
                nc.sync.dma_start(t3[:], wd3[k * 128:(k + 1) * 128, :])
                wd3_t.append(t3)
            wd4_t = []
            for k in range(2):
                t4 = cp.tile([128, 1], f32, tag=f"wd4{k}")
                nc.sync.dma_start(t4[:], wd4[k * 128:(k + 1) * 128, :])
                wd4_t.append(t4)
            bd_t = []
            for nm, hndl in (("bd1", bd1c), ("bd2", bd2c), ("bd3", bd3c)):
                t = cp.tile([128, 2], f32, tag=nm)
                nc.sync.dma_start(t[:], hndl[:])
                bd_t.append(t)
            bd4_t = cp.tile([1, 1], f32, tag="bd4")
            nc.sync.dma_start(bd4_t[:], bd4[:])
            ones8 = cp.tile([8, 1], f32, tag="ones8")
            nc.vector.memset(ones8[:], 1.0)

            def leaky(dst_ap, ps_ap, bias_ap, mtag):
                t0 = wk.tile([128, 1], f32, tag=f"lk0{mtag}")
                nc.vector.tensor_add(t0[:], ps_ap, bias_ap)
                t1 = wk.tile([128, 1], f32, tag=f"lk1{mtag}")
                nc.vector.tensor_scalar_mul(t1[:], t0[:], 0.1)
                nc.vector.tensor_max(dst_ap, t0[:], t1[:])

            # ================= GCN =================
            if mode in ("head", "head_pe", "head_dma"):
                h3T = []
                for j in range(4):
                    t16 = wk.tile([128, S], f16, tag=f"h3T{j}")
                    nc.vector.memset(t16[:], 0.001)
                    h3T.append(t16)
            if mode not in ("head", "head_pe", "head_dma"):
                # ---- layer 1 (local 128 dst nodes) ----
                ps_a1 = pp.tile([128, S], f32, tag="ps_sm")
                for k in range(8):
                    nc.tensor.matmul(ps_a1[:], xk_t[k][:], ats_t[k][:],
                                     start=(k == 0), stop=(k == 7))
                a1 = wk.tile([128, S], f32, tag="a1")
                nc.vector.tensor_copy(a1[:], ps_a1[:])
                ps_h1 = pp.tile([128, D], f32, tag="ps_h")
                nc.tensor.matmul(ps_h1[:], a1[:], w1_t[:], start=True, stop=True)
                hb1 = wk.tile([128, D], f32, tag="hb")
                nc.vector.tensor_add(hb1[:], ps_h1[:], b_t[0][:])
                hs1 = wk.tile([128, D], f32, tag="hs")
                nc.scalar.activation(hs1[:], hb1[:], Tanh)

                # AllGather h1
                cc1i = dp.tile([128, D], f32, tag="cc1i")
                nc.sync.dma_start(cc1i[:], hs1[:])
                cc1o = dp.tile([P, D], f32, tag="cc1o", addr_space="Shared")
                nc.gpsimd.collective_compute(
                    "AllGather", Bypass, replica_groups=RG,
                    ins=[cc1i.opt()], outs=[cc1o.opt()])
                h1_t = []
                for q in range(8):
                    t = hp.tile([128, D], f32, tag="hfull")
                    nc.sync.dma_start(t[:], cc1o[q * 128:(q + 1) * 128, :])
                    h1_t.append(t)

                def gcn_layer(h_in_t, w_chunks, b_tile, lname):
                    # agg^T slices: [feat_chunk m partitions, S nodes]
                    a2 = wk.tile([128, 4 * S], f32, tag=f"agg{lname}")
                    for m in range(4):
                        ps = pp.tile([128, S], f32, tag="ps_sm")
                        for k in range(8):
                            nc.tensor.matmul(
                                ps[:], h_in_t[k][:, m * 128:(m + 1) * 128],
                                ats_t[k][:], start=(k == 0), stop=(k == 7))
                        nc.vector.tensor_copy(a2[:, m * S:(m + 1) * S], ps[:])
                    ps_h = pp.tile([128, D], f32, tag="ps_h")
                    for m in range(4):
                        nc.tensor.matmul(ps_h[:], a2[:, m * S:(m + 1) * S],
                                         w_chunks[m][:],
                                         start=(m == 0), stop=(m == 3))
                    hb = wk.tile([128, D], f32, tag="hb")
                    nc.vector.tensor_add(hb[:], ps_h[:], b_tile[:])
                    hs = wk.tile([128, D], f32, tag="hs")
                    nc.scalar.activation(hs[:], hb[:], Tanh)
                    return hs

                # ---- layer 2 ----
                hs2 = gcn_layer(h1_t, w2_t, b_t[1], "l2")
                cc2i = dp.tile([128, D], f32, tag="cc2i")
                nc.sync.dma_start(cc2i[:], hs2[:])
                cc2o = dp.tile([P, D], f32, tag="cc2o", addr_space="Shared")
                nc.gpsimd.collective_compute(
                    "AllGather", Bypass, replica_groups=RG,
                    ins=[cc2i.opt()], outs=[cc2o.opt()])
                h2_t = []
                for q in range(8):
                    t = hp.tile([128, D], f32, tag="hfull")
                    nc.sync.dma_start(t[:], cc2o[q * 128:(q + 1) * 128, :])
                    h2_t.append(t)

                # ---- layer 3 (local slice only; no gather) ----
                hs3 = gcn_layer(h2_t, w3_t, b_t[2], "l3")

                # ---- transpose local h3 [S,D] -> 4x [128, S] fp16 ----
                h3T = []
                for j in range(4):
                    pst = pp.tile([128, S], f32, tag="ps_sm")
                    nc.tensor.transpose(pst[:], hs3[:, j * 128:(j + 1) * 128],
                                        id_t[:])
                    t16 = wk.tile([128, S], f16, tag=f"h3T{j}")
                    nc.vector.tensor_copy(t16[:], pst[:])
                    h3T.append(t16)

            if mode == "gcn":
                out_sb = wk.tile([1, 1], f32, tag="out_sb")
                nc.vector.tensor_copy(out_sb[:], hs3[:1, :1])
                nc.sync.dma_start(out[:], out_sb[:])
            else:
                # ================= dense head =================
                ps_y1 = pacc.tile([1, H], f32, tag="ps_y1")
                for g in range(N_SLAB):
                    if mode == "head_pe":
                        slab = static_slabs[g % 8]
                    else:
                        slab = wpool.tile([128, SLAB_W], f16, tag="slab")
                        eng = nc.sync if g % 2 == 0 else nc.scalar
                        eng.dma_start(slab[:], wd1s[:, g * SLAB_W:(g + 1) * SLAB_W])
                    if mode == "head_dma":
                        continue
                    for t in range(SLAB_CH):
                        c = g * SLAB_CH + t
                        i, j = c // 4, c % 4
                        nc.tensor.matmul(
                            ps_y1[:], h3T[j][:, i:i + 1],
                            slab[:, t * H:(t + 1) * H],
                            start=(c == 0), stop=(c == KCH - 1))
                if mode == "head_dma":
                    nc.vector.memset(ps_y1[:], 0.0)
                y1p = wk.tile([1, H], f32, tag="y1p")
                nc.vector.tensor_copy(y1p[:], ps_y1[:])

                # AllGather partials -> [8, H], then sum over partition dim
                ccyi = dp.tile([1, H], f32, tag="ccyi")
                nc.sync.dma_start(ccyi[:], y1p[:])
                ccyo = dp.tile([8, H], f32, tag="ccyo", addr_space="Shared")
                nc.gpsimd.collective_compute(
                    "AllGather", Bypass, replica_groups=RG,
                    ins=[ccyi.opt()], outs=[ccyo.opt()])
                y1g = wk.tile([8, H], f32, tag="y1g")
                nc.sync.dma_start(y1g[:], ccyo[:])

                y1c = wk.tile([128, 2], f32, tag="y1c")
                for m in range(2):
                    ps = pp.tile([128, 1], f32, tag="ps_small")
                    nc.tensor.matmul(ps[:], y1g[:, m * 128:(m + 1) * 128],
                                     ones8[:], start=True, stop=True)
                    leaky(y1c[:, m:m + 1], ps[:], bd_t[0][:, m:m + 1], f"y1{m}")

                def dense(y_in, w_chunks, bias, oname):
                    y_out = wk.tile([128, 2], f32, tag=oname)
                    for m in range(2):
                        ps = pp.tile([128, 1], f32, tag="ps_small")
                        for k in range(2):
                            nc.tensor.matmul(
                                ps[:], w_chunks[k][:, m * 128:(m + 1) * 128],
                                y_in[:, k:k + 1], start=(k == 0), stop=(k == 1))
                        leaky(y_out[:, m:m + 1], ps[:], bias[:, m:m + 1],
                              f"{oname}{m}")
                    return y_out

                y2c = dense(y1c, wd2_t, bd_t[1], "y2c")
                y3c = dense(y2c, wd3_t, bd_t[2], "y3c")

                ps_o = pp.tile([1, 1], f32, tag="ps_small")
                for k in range(2):
                    nc.tensor.matmul(ps_o[:], wd4_t[k][:],
                                     y3c[:, k:k + 1], start=(k == 0), stop=(k == 1))
                out_sb = wk.tile([1, 1], f32, tag="out_sb")
                nc.vector.tensor_add(out_sb[:], ps_o[:], bd4_t[:])
                nc.sync.dma_start(out[:], out_sb[:])

    nc.compile()
    return nc


def _get_nc():
    global _NC
    if _NC is None:
        _NC = _build_nc()
    return _NC


def make_in_maps(inputs):
    """Host-side sharding / preprocessing. Returns per-core input dicts."""
    x = np.ascontiguousarray(np.asarray(inputs["x"], dtype=np.float32))
    ei = np.asarray(inputs["edge_index"])
    W1 = np.asarray(inputs["W1"], np.float32)
    W2 = np.asarray(inputs["W2"], np.float32)
    W3 = np.asarray(inputs["W3"], np.float32)
    b1 = np.asarray(inputs["b1"], np.float32)
    b2 = np.asarray(inputs["b2"], np.float32)
    b3 = np.asarray(inputs["b3"], np.float32)
    Wd1 = np.asarray(inputs["Wd1"], np.float32)
    Wd2 = np.asarray(inputs["Wd2"], np.float32)
    Wd3 = np.asarray(inputs["Wd3"], np.float32)
    Wd4 = np.asarray(inputs["Wd4"], np.float32)
    bd1 = np.asarray(inputs["bd1"], np.float32)
    bd2 = np.asarray(inputs["bd2"], np.float32)
    bd3 = np.asarray(inputs["bd3"], np.float32)
    bd4 = np.asarray(inputs["bd4"], np.float32)

    # normalized adjacency with self loops (GCNConv)
    src = ei[0].astype(np.int64)
    dst = ei[1].astype(np.int64)
    loop = np.arange(N, dtype=np.int64)
    s_all = np.concatenate([src, loop])
    d_all = np.concatenate([dst, loop])
    deg = np.bincount(d_all, minlength=N).astype(np.float32)
    dinv = np.where(deg > 0, 1.0 / np.sqrt(deg), 0.0).astype(np.float32)
    wnorm = dinv[s_all] * dinv[d_all]
    A = np.zeros((N, N), np.float32)
    np.add.at(A, (d_all, s_all), wnorm)
    AT = np.zeros((P, P), np.float32)
    AT[:N, :N] = A.T

    xk = np.zeros((P, F), np.float32)
    xk[:N] = x

    Wd1p = np.zeros((P * D, H), np.float16)
    Wd1p[:N * D] = Wd1.astype(np.float16)

    bb = lambda b: np.ascontiguousarray(np.broadcast_to(b[None, :], (128, b.shape[0])))
    bdc = lambda b: np.ascontiguousarray(b.reshape(2, 128).T)

    common = {
        "xk": xk,
        "w1": W1, "w2": W2, "w3": W3,
        "b1bc": bb(b1), "b2bc": bb(b2), "b3bc": bb(b3),
        "ident": np.eye(128, dtype=np.float32),
        "wd2": Wd2, "wd3": Wd3, "wd4": Wd4.reshape(H, 1),
        "bd1c": bdc(bd1), "bd2c": bdc(bd2), "bd3c": bdc(bd3),
        "bd4": bd4.reshape(1, 1),
    }

    in_maps = []
    rows_per_core = P * D // N_CORES  # 65536
    for r in range(N_CORES):
        sl = Wd1p[r * rows_per_core:(r + 1) * rows_per_core]
        # row = 512*i + 128*j + p  ->  [p, i, j, n] layout
        wd1s = np.ascontiguousarray(
            sl.reshape(S, 4, 128, H).transpose(2, 0, 1, 3).reshape(128, KCH * H))
        m = dict(common)
        m["ats"] = np.ascontiguousarray(AT[:, r * S:(r + 1) * S])
        m["wd1s"] = wd1s
        in_maps.append(m)
    return in_maps


def kernel(**inputs):
    from concourse.bass_utils import run_bass_kernel_spmd
    in_maps = make_in_maps(inputs)
    nc = _get_nc()
    res = run_bass_kernel_spmd(nc, in_maps, core_ids=list(range(N_CORES)))
    return np.asarray(res.results[0]["out"], np.float32).reshape(1)



# revision 6
# speedup vs baseline: 1.5248x; 1.5248x over previous
"""DeepHamCritic (3x GCNConv + dense head) on 8 trn2 NeuronCores.

v2 strategy (collective-minimal):
  - GCN layers 1+2 computed REPLICATED on every core (all 1024 padded
    nodes, fp16 matmuls vs the dense normalized adjacency) -- this
    removes the two inter-layer AllGathers entirely (each measured
    ~45us on this fabric vs ~25us of extra PE work).
  - Layer 3 computed only for the core's local 125 destination nodes,
    directly in TRANSPOSED form (h3T[j] = [feat 128, node 125]) so the
    dense head needs no PE transpose step.
  - Dense head: Wd1 [512000,256] row-sharded by node (125 nodes = 64000
    rows/core, fp16), streamed through SBUF slabs on a single DMA queue
    (one queue already saturates ~390GB/s; more queues don't add BW)
    and consumed by a PE matvec accumulated in PSUM. Slab DMAs are
    issued at program start so they overlap the whole GCN phase.
  - One tiny AllGather of the [1,256] partials at the end (the only
    collective), then the small Wd2/Wd3/Wd4 layers replicated.
"""

import numpy as np

N_CORES = 8
N = 1000          # real nodes
P = 1024          # padded nodes for GCN grid
NL = 125          # real nodes per core (head shard)
F = 128           # input features
D = 512           # GCN hidden
H = 256           # dense hidden
KCH = NL * 4      # 500 f-chunks of 128 per core
SLAB_CH = 32      # chunks per DMA slab
N_SLAB = (KCH + SLAB_CH - 1) // SLAB_CH   # 16 (last partial)
SLAB_W = SLAB_CH * H                      # 8192 fp16 cols per slab
WSLAB_BUFS = 6

# c16a columns: atsT | xk | w1 | brows(row0=b1,row1=b2)
C16A_ATST = 0
C16A_XK = 8192
C16A_W1 = 9216
C16A_BROW = 9728
C16A_BROW2 = 10240
C16A_W = 10752
# c16b columns: w2 | w3 | atsL
C16B_W2 = 0
C16B_W3 = 2048
C16B_ATSL = 4096
C16B_W = 5120
# c32 columns: wd2 | wd3 | wd4 | bd1 | bd2 | bd3 | bd4 | b3col
C32_WD2 = 0
C32_WD3 = 512
C32_WD4 = 1024
C32_BD1 = 1026
C32_BD2 = 1028
C32_BD3 = 1030
C32_BD4 = 1032
C32_B3C = 1033
C32_W = 1037

_NC = {}


def _build_nc(reps=1, mode="full"):
    import concourse.bacc as bacc
    import concourse.mybir as mybir
    import concourse.tile as tile

    f32 = mybir.dt.float32
    f16 = mybir.dt.float16
    RG = [list(range(N_CORES))]

    nc = bacc.Bacc("TRN2", target_bir_lowering=False, debug=False,
                   num_devices=N_CORES)

    c16a = nc.dram_tensor("c16a", [128, C16A_W], f16, kind="ExternalInput")
    c16b = nc.dram_tensor("c16b", [128, C16B_W], f16, kind="ExternalInput")
    c32 = nc.dram_tensor("c32", [128, C32_W], f32, kind="ExternalInput")
    wd1s = nc.dram_tensor("wd1s", [128, KCH * H], f16, kind="ExternalInput")
    out = nc.dram_tensor("out", [1, 1], f32, kind="ExternalOutput")

    Tanh = mybir.ActivationFunctionType.Tanh
    Lrelu = mybir.ActivationFunctionType.Lrelu
    Bypass = mybir.AluOpType.bypass

    do_gcn = mode in ("full", "gcn")
    do_head_pe = mode in ("full", "gcn_head", "head_pe")
    do_slab_dma = mode in ("full", "head_dma")
    do_tail = mode == "full"

    with tile.TileContext(nc) as tc:
        with (
            tc.tile_pool(name="wslab", bufs=WSLAB_BUFS) as wpool,
            tc.tile_pool(name="const", bufs=1) as cp,
            tc.tile_pool(name="hbuf", bufs=1) as hp,
            tc.tile_pool(name="work", bufs=2) as wk,
            tc.tile_pool(name="psum", bufs=2, space="PSUM") as pp,
            tc.tile_pool(name="ps3", bufs=2, space="PSUM") as p3,
            tc.tile_pool(name="psacc", bufs=1, space="PSUM") as pacc,
            tc.tile_pool(name="dram", bufs=1, space="DRAM") as dp,
        ):
          static_slabs = None
          if mode == "head_pe":
            static_slabs = []
            for sidx in range(4):
                st = cp.tile([128, SLAB_W], f16, tag=f"sslab{sidx}")
                nc.vector.memset(st[:], 0.001)
                static_slabs.append(st)
          for _rep in range(reps):
            # ---- all big DMAs on the sync queue, consts first ----
            ca = cp.tile([128, C16A_W], f16, tag="c16a")
            cb = cp.tile([128, C16B_W], f16, tag="c16b")
            cc = cp.tile([128, C32_W], f32, tag="c32")
            if do_gcn or do_tail:
                nc.sync.dma_start(ca[:], c16a[:])
                nc.sync.dma_start(cc[:], c32[:])
                nc.sync.dma_start(cb[:], c16b[:])
            slabs = []
            if do_slab_dma:
                for g in range(N_SLAB):
                    w = min(SLAB_W, KCH * H - g * SLAB_W)
                    t = wpool.tile([128, SLAB_W], f16, tag="slab")
                    nc.sync.dma_start(t[:, :w],
                                      wd1s[:, g * SLAB_W:g * SLAB_W + w])
                    slabs.append(t)
            elif do_head_pe and mode == "head_pe":
                slabs = [static_slabs[g % 4] for g in range(N_SLAB)]

            ones1 = cp.tile([1, 128], f16, tag="ones1")
            nc.vector.memset(ones1[:], 1.0)
            ones8 = cp.tile([8, 1], f32, tag="ones8")
            nc.vector.memset(ones8[:], 1.0)

            h3T = None
            if do_gcn:
                # ============ GCN layer 1 (replicated, agg-first) ====
                # aggT [f 128, dst 1024] = sum_k xk_k^T @ atsT_k
                ps_h1 = [pp.tile([128, 512], f32, tag="ps_ag", name=f"ps_h1_{hh}")
                         for hh in range(2)]
                for k in range(8):
                    for hh in range(2):
                        nc.tensor.matmul(
                            ps_h1[hh][:],
                            ca[:, C16A_XK + k * 128:C16A_XK + (k + 1) * 128],
                            ca[:, k * 1024 + hh * 512:k * 1024 + (hh + 1) * 512],
                            start=(k == 0), stop=(k == 7))
                agg1T = hp.tile([128, 1024], f16, tag="agg1T")
                for hh in range(2):
                    nc.vector.tensor_copy(agg1T[:, hh * 512:(hh + 1) * 512],
                                          ps_h1[hh][:])
                # h1[n,512] = tanh(agg1[n,:] @ W1 + b1), n-block at a time
                h1 = hp.tile([128, 8 * 512], f16, tag="h1")
                for n in range(8):
                    ps = pp.tile([128, 512], f32, tag="ps_tr")
                    nc.tensor.matmul(ps[:], ones1[:],
                                     ca[0:1, C16A_BROW:C16A_BROW + 512],
                                     start=True, stop=False)
                    nc.tensor.matmul(ps[:],
                                     agg1T[:, n * 128:(n + 1) * 128],
                                     ca[:, C16A_W1:C16A_W1 + 512],
                                     start=False, stop=True)
                    nc.scalar.activation(h1[:, n * 512:(n + 1) * 512], ps[:],
                                         Tanh)

                # ============ GCN layer 2 (replicated) ===============
                agg2T = hp.tile([128, 4 * 1024], f16, tag="agg2T")
                for m in range(4):
                    psm = [pp.tile([128, 512], f32, tag="ps_ag", name=f"psm{hh}")
                           for hh in range(2)]
                    for k in range(8):
                        for hh in range(2):
                            nc.tensor.matmul(
                                psm[hh][:],
                                h1[:, k * 512 + m * 128:k * 512 + (m + 1) * 128],
                                ca[:, k * 1024 + hh * 512:k * 1024 + (hh + 1) * 512],
                                start=(k == 0), stop=(k == 7))
                    for hh in range(2):
                        nc.vector.tensor_copy(
                            agg2T[:, m * 1024 + hh * 512:m * 1024 + (hh + 1) * 512],
                            psm[hh][:])
                h2 = hp.tile([128, 8 * 512], f16, tag="h2")
                for n in range(8):
                    ps = pp.tile([128, 512], f32, tag="ps_tr")
                    nc.tensor.matmul(ps[:], ones1[:],
                                     ca[0:1, C16A_BROW2:C16A_BROW2 + 512],
                                     start=True, stop=False)
                    for m in range(4):
                        nc.tensor.matmul(
                            ps[:],
                            agg2T[:, m * 1024 + n * 128:m * 1024 + (n + 1) * 128],
                            cb[:, C16B_W2 + m * 512:C16B_W2 + (m + 1) * 512],
                            start=False, stop=(m == 3))
                    nc.scalar.activation(h2[:, n * 512:(n + 1) * 512], ps[:],
                                         Tanh)

                # ======= GCN layer 3 (local 125 dst, transposed out) =
                a3T = hp.tile([128, 4 * 128], f16, tag="a3T")
                for m in range(4):
                    ps = p3.tile([128, 128], f32, tag="ps_sm")
                    for k in range(8):
                        nc.tensor.matmul(
                            ps[:],
                            h2[:, k * 512 + m * 128:k * 512 + (m + 1) * 128],
                            cb[:, C16B_ATSL + k * 128:C16B_ATSL + (k + 1) * 128],
                            start=(k == 0), stop=(k == 7))
                    nc.vector.tensor_copy(a3T[:, m * 128:(m + 1) * 128], ps[:])
                # h3T[j] [d 128, n 128] = tanh(sum_m w3(m,j)^T @ a3T_m + b3)
                h3T = []
                for j in range(4):
                    ps = p3.tile([128, 128], f32, tag="ps_sm")
                    for m in range(4):
                        nc.tensor.matmul(
                            ps[:],
                            cb[:, C16B_W3 + m * 512 + j * 128:
                                C16B_W3 + m * 512 + (j + 1) * 128],
                            a3T[:, m * 128:(m + 1) * 128],
                            start=(m == 0), stop=(m == 3))
                    t = wk.tile([128, 128], f16, tag=f"h3T{j}")
                    nc.scalar.activation(t[:], ps[:], Tanh,
                                         bias=cc[:, C32_B3C + j:C32_B3C + j + 1])
                    h3T.append(t)
            elif do_head_pe:
                h3T = []
                for j in range(4):
                    t = wk.tile([128, 128], f16, tag=f"h3T{j}")
                    nc.vector.memset(t[:], 0.001)
                    h3T.append(t)

            if mode == "gcn":
                out_sb = wk.tile([1, 1], f32, tag="out_sb")
                nc.vector.tensor_copy(out_sb[:], h3T[0][:1, :1])
                nc.sync.dma_start(out[:], out_sb[:])
                continue
            if mode == "head_dma":
                out_sb = wk.tile([1, 1], f32, tag="out_sb")
                nc.vector.tensor_copy(out_sb[:], slabs[-1][:1, :1])
                nc.sync.dma_start(out[:], out_sb[:])
                continue

            # ================= dense head matvec =================
            ps_y1 = pacc.tile([1, H], f32, tag="ps_y1")
            for g in range(N_SLAB):
                slab = slabs[g]
                nch = min(SLAB_CH, KCH - g * SLAB_CH)
                for t in range(nch):
                    c = g * SLAB_CH + t
                    i, j = c // 4, c % 4
                    nc.tensor.matmul(
                        ps_y1[:], h3T[j][:, i:i + 1],
                        slab[:, t * H:(t + 1) * H],
                        start=(c == 0), stop=(c == KCH - 1))
            y1p = wk.tile([1, H], f32, tag="y1p")
            nc.vector.tensor_copy(y1p[:], ps_y1[:])

            if mode == "head_pe":
                out_sb = wk.tile([1, 1], f32, tag="out_sb")
                nc.vector.tensor_copy(out_sb[:], y1p[:1, :1])
                nc.sync.dma_start(out[:], out_sb[:])
                continue

            # ---- the only collective: gather [1,256] partials ----
            ccyi = dp.tile([1, H], f32, tag="ccyi")
            nc.sync.dma_start(ccyi[:], y1p[:])
            ccyo = dp.tile([8, H], f32, tag="ccyo", addr_space="Shared")
            nc.gpsimd.collective_compute(
                "AllGather", Bypass, replica_groups=RG,
                ins=[ccyi.opt()], outs=[ccyo.opt()])
            y1g = wk.tile([8, H], f32, tag="y1g")
            nc.sync.dma_start(y1g[:], ccyo[:])

            def leaky(dst_ap, ps_ap, bias_ap, mtag):
                t0 = wk.tile([128, 1], f32, tag=f"lk0{mtag}", name="t0")
                nc.vector.tensor_add(t0[:], ps_ap, bias_ap)
                t1 = wk.tile([128, 1], f32, tag=f"lk1{mtag}", name="t1")
                nc.vector.tensor_scalar_mul(t1[:], t0[:], 0.1)
                nc.vector.tensor_max(dst_ap, t0[:], t1[:])

            # sum partials + bias + leaky
            y1c = wk.tile([128, 2], f32, tag="y1c")
            for m in range(2):
                ps = p3.tile([128, 1], f32, tag="ps_sm")
                nc.tensor.matmul(ps[:], y1g[:, m * 128:(m + 1) * 128],
                                 ones8[:], start=True, stop=True)
                leaky(y1c[:, m:m + 1], ps[:],
                      cc[:, C32_BD1 + m:C32_BD1 + m + 1], f"y1{m}")

            def dense(y_in, wcol, bcol, oname):
                y_out = wk.tile([128, 2], f32, tag=oname)
                for m in range(2):
                    ps = p3.tile([128, 1], f32, tag="ps_sm")
                    for k in range(2):
                        nc.tensor.matmul(
                            ps[:],
                            cc[:, wcol + k * 256 + m * 128:
                                wcol + k * 256 + (m + 1) * 128],
                            y_in[:, k:k + 1], start=(k == 0), stop=(k == 1))
                    leaky(y_out[:, m:m + 1], ps[:],
                          cc[:, bcol + m:bcol + m + 1], f"{oname}{m}")
                return y_out

            y2c = dense(y1c, C32_WD2, C32_BD2, "y2c")
            y3c = dense(y2c, C32_WD3, C32_BD3, "y3c")

            ps_o = p3.tile([1, 1], f32, tag="ps_sm")
            for k in range(2):
                nc.tensor.matmul(ps_o[:],
                                 cc[:, C32_WD4 + k:C32_WD4 + k + 1],
                                 y3c[:, k:k + 1], start=(k == 0), stop=(k == 1))
            out_sb = wk.tile([1, 1], f32, tag="out_sb")
            nc.vector.tensor_add(out_sb[:], ps_o[:], cc[0:1, C32_BD4:C32_BD4 + 1])
            nc.sync.dma_start(out[:], out_sb[:])

    nc.compile()
    return nc


def _get_nc():
    if "full" not in _NC:
        _NC["full"] = _build_nc()
    return _NC["full"]


def make_in_maps(inputs):
    """Host-side sharding / preprocessing. Returns per-core input dicts."""
    x = np.asarray(inputs["x"], dtype=np.float32)
    ei = np.asarray(inputs["edge_index"])
    W1 = np.asarray(inputs["W1"], np.float32)
    W2 = np.asarray(inputs["W2"], np.float32)
    W3 = np.asarray(inputs["W3"], np.float32)
    b1 = np.asarray(inputs["b1"], np.float32)
    b2 = np.asarray(inputs["b2"], np.float32)
    b3 = np.asarray(inputs["b3"], np.float32)
    Wd1 = np.asarray(inputs["Wd1"], np.float32)
    Wd2 = np.asarray(inputs["Wd2"], np.float32)
    Wd3 = np.asarray(inputs["Wd3"], np.float32)
    Wd4 = np.asarray(inputs["Wd4"], np.float32)
    bd1 = np.asarray(inputs["bd1"], np.float32)
    bd2 = np.asarray(inputs["bd2"], np.float32)
    bd3 = np.asarray(inputs["bd3"], np.float32)
    bd4 = np.asarray(inputs["bd4"], np.float32)

    # normalized adjacency with self loops (GCNConv): A[dst, src]
    src = ei[0].astype(np.int64)
    dst = ei[1].astype(np.int64)
    loop = np.arange(N, dtype=np.int64)
    s_all = np.concatenate([src, loop])
    d_all = np.concatenate([dst, loop])
    deg = np.bincount(d_all, minlength=N).astype(np.float32)
    dinv = np.where(deg > 0, 1.0 / np.sqrt(deg), 0.0).astype(np.float32)
    wnorm = dinv[s_all] * dinv[d_all]
    A = np.zeros((N, N), np.float32)
    np.add.at(A, (d_all, s_all), wnorm)

    # atsT[p, k*1024 + d] = A[d, k*128 + p]  (A^T in [src_p, src_blk, dst])
    AT = np.zeros((P, P), np.float32)
    AT[:N, :N] = A.T
    atsT = AT.reshape(8, 128, P).transpose(1, 0, 2).reshape(128, 8 * P)

    xkp = np.zeros((P, F), np.float32)
    xkp[:N] = x
    xk = xkp.reshape(8, 128, F).transpose(1, 0, 2).reshape(128, 8 * F)

    c16a = np.zeros((128, C16A_W), np.float16)
    c16a[:, C16A_ATST:C16A_ATST + 8 * P] = atsT.astype(np.float16)
    c16a[:, C16A_XK:C16A_XK + 8 * F] = xk.astype(np.float16)
    c16a[:, C16A_W1:C16A_W1 + D] = W1.astype(np.float16)
    c16a[0, C16A_BROW:C16A_BROW + D] = b1.astype(np.float16)
    c16a[0, C16A_BROW2:C16A_BROW2 + D] = b2.astype(np.float16)

    # w2/w3 as [p, m*512 + d] = W[m*128+p, d]
    w2l = W2.reshape(4, 128, D).transpose(1, 0, 2).reshape(128, 4 * D)
    w3l = W3.reshape(4, 128, D).transpose(1, 0, 2).reshape(128, 4 * D)

    c32 = np.zeros((128, C32_W), np.float32)
    c32[:, C32_WD2:C32_WD2 + 512] = Wd2.reshape(2, 128, H).transpose(
        1, 0, 2).reshape(128, 512)
    c32[:, C32_WD3:C32_WD3 + 512] = Wd3.reshape(2, 128, H).transpose(
        1, 0, 2).reshape(128, 512)
    c32[:, C32_WD4:C32_WD4 + 2] = Wd4.reshape(2, 128).T
    c32[:, C32_BD1:C32_BD1 + 2] = bd1.reshape(2, 128).T
    c32[:, C32_BD2:C32_BD2 + 2] = bd2.reshape(2, 128).T
    c32[:, C32_BD3:C32_BD3 + 2] = bd3.reshape(2, 128).T
    c32[0, C32_BD4] = bd4[0]
    c32[:, C32_B3C:C32_B3C + 4] = b3.reshape(4, 128).T

    in_maps = []
    for r in range(N_CORES):
        c16b = np.zeros((128, C16B_W), np.float16)
        c16b[:, C16B_W2:C16B_W2 + 4 * D] = w2l.astype(np.float16)
        c16b[:, C16B_W3:C16B_W3 + 4 * D] = w3l.astype(np.float16)
        # atsL[p, k*128 + dd] = A[r*125 + dd, k*128 + p]
        atsL = np.zeros((128, 8 * 128), np.float16)
        loc = AT[:, r * NL:(r + 1) * NL].reshape(8, 128, NL).transpose(
            1, 0, 2)  # [p, k, dd]
        atsLf = np.zeros((128, 8, 128), np.float32)
        atsLf[:, :, :NL] = loc
        atsL[:] = atsLf.reshape(128, 8 * 128).astype(np.float16)
        c16b[:, C16B_ATSL:C16B_ATSL + 8 * 128] = atsL

        sl = Wd1[r * NL * D:(r + 1) * NL * D]  # [64000, 256]
        wd1 = np.ascontiguousarray(
            sl.reshape(NL, 4, 128, H).transpose(2, 0, 1, 3).reshape(
                128, KCH * H)).astype(np.float16)
        in_maps.append({"c16a": c16a, "c16b": c16b, "c32": c32,
                        "wd1s": wd1})
    return in_maps


def kernel(**inputs):
    from concourse.bass_utils import run_bass_kernel_spmd
    in_maps = make_in_maps(inputs)
    nc = _get_nc()
    res = run_bass_kernel_spmd(nc, in_maps, core_ids=list(range(N_CORES)))
    return np.asarray(res.results[0]["out"], np.float32).reshape(1)


# revision 9
# speedup vs baseline: 1.6623x; 1.0902x over previous
"""DeepHamCritic (3x GCNConv + dense head) on 8 trn2 NeuronCores.

v2 strategy (collective-minimal):
  - GCN layers 1+2 computed REPLICATED on every core (all 1024 padded
    nodes, fp16 matmuls vs the dense normalized adjacency) -- this
    removes the two inter-layer AllGathers entirely (each measured
    ~45us on this fabric vs ~25us of extra PE work).
  - Layer 3 computed only for the core's local 125 destination nodes,
    directly in TRANSPOSED form (h3T[j] = [feat 128, node 125]) so the
    dense head needs no PE transpose step.
  - Dense head: Wd1 [512000,256] row-sharded by node (125 nodes = 64000
    rows/core, fp16), streamed through SBUF slabs on a single DMA queue
    (one queue already saturates ~390GB/s; more queues don't add BW)
    and consumed by a PE matvec accumulated in PSUM. Slab DMAs are
    issued at program start so they overlap the whole GCN phase.
  - One tiny AllGather of the [1,256] partials at the end (the only
    collective), then the small Wd2/Wd3/Wd4 layers replicated.
"""

import numpy as np

N_CORES = 8
N = 1000          # real nodes
P = 1024          # padded nodes for GCN grid
NL = 125          # real nodes per core (head shard)
F = 128           # input features
D = 512           # GCN hidden
H = 256           # dense hidden
KCH = NL * 4      # 500 real f-chunks of 128 per core
JCH = 126         # chunks per j-block (125 real + 1 zero pad)
KCHP = JCH * 4    # 504 padded chunks, j-major layout
NPAIR = KCHP // 2  # 252 paired matmuls
SLAB_CH = 32      # chunks per DMA slab
N_SLAB = (KCHP + SLAB_CH - 1) // SLAB_CH  # 16 (last partial)
SLAB_W = SLAB_CH * H                      # 8192 fp16 cols per slab
WSLAB_BUFS = 6

# c16a columns: atsT | xk | w1
C16A_ATST = 0
C16A_XK = 8192
C16A_W1 = 9216
C16A_W = 9728
# c16b columns: w2 | w3 | atsL
C16B_W2 = 0
C16B_W3 = 2048
C16B_ATSL = 4096
C16B_W = 5120
# c32 columns: wd2 | wd3 | wd4 | bd1 | bd2 | bd3 | bd4 | b3col
C32_WD2 = 0
C32_WD3 = 512
C32_WD4 = 1024
C32_BD1 = 1026
C32_BD2 = 1028
C32_BD3 = 1030
C32_BD4 = 1032
C32_B3C = 1033
C32_B1BC = 1037
C32_B2BC = 1549
C32_W = 2061

_NC = {}


def _build_nc(reps=1, mode="full"):
    import concourse.bacc as bacc
    import concourse.mybir as mybir
    import concourse.tile as tile

    f32 = mybir.dt.float32
    f16 = mybir.dt.float16
    RG = [list(range(N_CORES))]

    nc = bacc.Bacc("TRN2", target_bir_lowering=False, debug=False,
                   num_devices=N_CORES)

    c16a = nc.dram_tensor("c16a", [128, C16A_W], f16, kind="ExternalInput")
    c16b = nc.dram_tensor("c16b", [128, C16B_W], f16, kind="ExternalInput")
    c32 = nc.dram_tensor("c32", [128, C32_W], f32, kind="ExternalInput")
    wd1s = nc.dram_tensor("wd1s", [128, KCHP * H], f16, kind="ExternalInput")
    out = nc.dram_tensor("out", [1, 1], f32, kind="ExternalOutput")

    Tanh = mybir.ActivationFunctionType.Tanh
    Lrelu = mybir.ActivationFunctionType.Lrelu
    Bypass = mybir.AluOpType.bypass

    do_gcn = mode in ("full", "gcn")
    do_head_pe = mode in ("full", "gcn_head", "head_pe")
    do_slab_dma = mode in ("full", "head_dma")
    do_tail = mode == "full"

    with tile.TileContext(nc) as tc:
        with (
            tc.tile_pool(name="wslab", bufs=WSLAB_BUFS) as wpool,
            tc.tile_pool(name="const", bufs=1) as cp,
            tc.tile_pool(name="hbuf", bufs=1) as hp,
            tc.tile_pool(name="work", bufs=2) as wk,
            tc.tile_pool(name="psum", bufs=2, space="PSUM") as pp,
            tc.tile_pool(name="ps3", bufs=2, space="PSUM") as p3,
            tc.tile_pool(name="psacc", bufs=1, space="PSUM") as pacc,
            tc.tile_pool(name="dram", bufs=1, space="DRAM") as dp,
        ):
          static_slabs = None
          if mode == "head_pe":
            static_slabs = []
            for sidx in range(4):
                st = cp.tile([128, SLAB_W], f16, tag=f"sslab{sidx}")
                nc.vector.memset(st[:], 0.001)
                static_slabs.append(st)
          for _rep in range(reps):
            # ---- all big DMAs on the sync queue, consts first ----
            ca = cp.tile([128, C16A_W], f16, tag="c16a")
            cb = cp.tile([128, C16B_W], f16, tag="c16b")
            cc = cp.tile([128, C32_W], f32, tag="c32")
            if do_gcn or do_tail:
                nc.sync.dma_start(ca[:], c16a[:])
                nc.sync.dma_start(cc[:], c32[:])
                nc.sync.dma_start(cb[:], c16b[:])
            slabs = []
            if do_slab_dma:
                for g in range(N_SLAB):
                    w = min(SLAB_W, KCHP * H - g * SLAB_W)
                    t = wpool.tile([128, SLAB_W], f16, tag="slab")
                    nc.sync.dma_start(t[:, :w],
                                      wd1s[:, g * SLAB_W:g * SLAB_W + w])
                    slabs.append(t)
            elif do_head_pe and mode == "head_pe":
                slabs = [static_slabs[g % 4] for g in range(N_SLAB)]

            ones8 = cp.tile([8, 1], f32, tag="ones8")
            nc.vector.memset(ones8[:], 1.0)

            h3T = None
            if do_gcn:
                # ============ GCN layer 1 (replicated, agg-first) ====
                # aggT [f 128, dst 1024] = sum_k xk_k^T @ atsT_k
                ps_h1 = [pp.tile([128, 512], f32, tag="ps_ag", name=f"ps_h1_{hh}")
                         for hh in range(2)]
                for k in range(8):
                    for hh in range(2):
                        nc.tensor.matmul(
                            ps_h1[hh][:],
                            ca[:, C16A_XK + k * 128:C16A_XK + (k + 1) * 128],
                            ca[:, k * 1024 + hh * 512:k * 1024 + (hh + 1) * 512],
                            start=(k == 0), stop=(k == 7))
                agg1T = hp.tile([128, 1024], f16, tag="agg1T")
                for hh in range(2):
                    nc.vector.tensor_copy(agg1T[:, hh * 512:(hh + 1) * 512],
                                          ps_h1[hh][:])
                # h1[n,512] = tanh(agg1[n,:] @ W1 + b1), n-block at a time
                h1 = hp.tile([128, 8 * 512], f16, tag="h1")
                for n in range(8):
                    ps = pp.tile([128, 512], f32, tag="ps_tr")
                    nc.tensor.matmul(ps[:],
                                     agg1T[:, n * 128:(n + 1) * 128],
                                     ca[:, C16A_W1:C16A_W1 + 512],
                                     start=True, stop=True)
                    hb = wk.tile([128, 512], f32, tag="hb1")
                    nc.vector.tensor_add(hb[:], ps[:],
                                         cc[:, C32_B1BC:C32_B1BC + 512])
                    nc.scalar.activation(h1[:, n * 512:(n + 1) * 512], hb[:],
                                         Tanh)

                # ============ GCN layer 2 (replicated) ===============
                agg2T = hp.tile([128, 4 * 1024], f16, tag="agg2T")
                for m in range(4):
                    psm = [pp.tile([128, 512], f32, tag="ps_ag", name=f"psm{hh}")
                           for hh in range(2)]
                    for k in range(8):
                        for hh in range(2):
                            nc.tensor.matmul(
                                psm[hh][:],
                                h1[:, k * 512 + m * 128:k * 512 + (m + 1) * 128],
                                ca[:, k * 1024 + hh * 512:k * 1024 + (hh + 1) * 512],
                                start=(k == 0), stop=(k == 7))
                    for hh in range(2):
                        nc.vector.tensor_copy(
                            agg2T[:, m * 1024 + hh * 512:m * 1024 + (hh + 1) * 512],
                            psm[hh][:])
                h2 = hp.tile([128, 8 * 512], f16, tag="h2")
                for n in range(8):
                    ps = pp.tile([128, 512], f32, tag="ps_tr")
                    for m in range(4):
                        nc.tensor.matmul(
                            ps[:],
                            agg2T[:, m * 1024 + n * 128:m * 1024 + (n + 1) * 128],
                            cb[:, C16B_W2 + m * 512:C16B_W2 + (m + 1) * 512],
                            start=(m == 0), stop=(m == 3))
                    hb = wk.tile([128, 512], f32, tag="hb2")
                    nc.vector.tensor_add(hb[:], ps[:],
                                         cc[:, C32_B2BC:C32_B2BC + 512])
                    nc.scalar.activation(h2[:, n * 512:(n + 1) * 512], hb[:],
                                         Tanh)

                # ======= GCN layer 3 (local 125 dst, transposed out) =
                a3T = hp.tile([128, 4 * 128], f16, tag="a3T")
                for m in range(4):
                    ps = p3.tile([128, 128], f32, tag="ps_sm")
                    for k in range(8):
                        nc.tensor.matmul(
                            ps[:],
                            h2[:, k * 512 + m * 128:k * 512 + (m + 1) * 128],
                            cb[:, C16B_ATSL + k * 128:C16B_ATSL + (k + 1) * 128],
                            start=(k == 0), stop=(k == 7))
                    nc.vector.tensor_copy(a3T[:, m * 128:(m + 1) * 128], ps[:])
                # h3T[j] [d 128, n 128] = tanh(sum_m w3(m,j)^T @ a3T_m + b3)
                h3T = []
                for j in range(4):
                    ps = p3.tile([128, 128], f32, tag="ps_sm")
                    for m in range(4):
                        nc.tensor.matmul(
                            ps[:],
                            cb[:, C16B_W3 + m * 512 + j * 128:
                                C16B_W3 + m * 512 + (j + 1) * 128],
                            a3T[:, m * 128:(m + 1) * 128],
                            start=(m == 0), stop=(m == 3))
                    t = wk.tile([128, 128], f16, tag=f"h3T{j}")
                    nc.scalar.activation(t[:], ps[:], Tanh,
                                         bias=cc[:, C32_B3C + j:C32_B3C + j + 1])
                    h3T.append(t)
            elif do_head_pe:
                h3T = []
                for j in range(4):
                    t = wk.tile([128, 128], f16, tag=f"h3T{j}")
                    nc.vector.memset(t[:], 0.001)
                    h3T.append(t)

            if mode == "gcn":
                out_sb = wk.tile([1, 1], f32, tag="out_sb")
                nc.vector.tensor_copy(out_sb[:], h3T[0][:1, :1])
                nc.sync.dma_start(out[:], out_sb[:])
                continue
            if mode == "head_dma":
                out_sb = wk.tile([1, 1], f32, tag="out_sb")
                nc.vector.tensor_copy(out_sb[:], slabs[-1][:1, :1])
                nc.sync.dma_start(out[:], out_sb[:])
                continue

            # ============ dense head matvec (paired chunks) ============
            ps_y = [pacc.tile([2, 512], f32, tag=f"ps_y{b}", name=f"ps_y{b}")
                    for b in range(2)]
            for g in range(N_SLAB):
                slab = slabs[g]
                npr = min(SLAB_CH // 2, NPAIR - g * (SLAB_CH // 2))
                for t2 in range(npr):
                    p = g * (SLAB_CH // 2) + t2
                    j, ip = p // 63, p % 63
                    b = p % 2
                    nc.tensor.matmul(
                        ps_y[b][:], h3T[j][:, 2 * ip:2 * ip + 2],
                        slab[:, t2 * 512:(t2 + 1) * 512],
                        start=(p < 2), stop=(p >= NPAIR - 2))
            # row 1 of each accumulator holds the other diag block; move
            # it to partition 0 via a [0,1]-selection matmul (partition-
            # base rule forbids direct partition-1 reads).
            e1 = cp.tile([2, 1], f32, tag="e1")
            nc.vector.memset(e1[:], 1.0)
            nc.vector.memset(e1[0:1, :], 0.0)
            ysb = []
            sel = []
            for b in range(2):
                t = wk.tile([2, 2 * H], f32, tag=f"ysb{b}", name=f"ysb{b}")
                nc.vector.tensor_copy(t[:], ps_y[b][:])
                ysb.append(t)
                s = p3.tile([1, 2 * H], f32, tag="ps_sm", name=f"sel{b}")
                nc.tensor.matmul(s[:], e1[:], t[:], start=True, stop=True)
                sel.append(s)
            ya = wk.tile([1, H], f32, tag="ya")
            nc.vector.tensor_add(ya[:], ysb[0][0:1, 0:H], ysb[1][0:1, 0:H])
            yb = wk.tile([1, H], f32, tag="yb")
            nc.vector.tensor_add(yb[:], ya[:], sel[0][0:1, H:2 * H])
            y1p = wk.tile([1, H], f32, tag="y1p")
            nc.vector.tensor_add(y1p[:], yb[:], sel[1][0:1, H:2 * H])

            if mode == "head_pe":
                out_sb = wk.tile([1, 1], f32, tag="out_sb")
                nc.vector.tensor_copy(out_sb[:], y1p[:1, :1])
                nc.sync.dma_start(out[:], out_sb[:])
                continue

            # ---- the only collective: gather [1,256] partials ----
            ccyi = dp.tile([1, H], f32, tag="ccyi")
            nc.sync.dma_start(ccyi[:], y1p[:])
            ccyo = dp.tile([8, H], f32, tag="ccyo", addr_space="Shared")
            nc.gpsimd.collective_compute(
                "AllGather", Bypass, replica_groups=RG,
                ins=[ccyi.opt()], outs=[ccyo.opt()])
            y1g = wk.tile([8, H], f32, tag="y1g")
            nc.sync.dma_start(y1g[:], ccyo[:])

            def leaky(dst_ap, ps_ap, bias_ap, mtag):
                t0 = wk.tile([128, 1], f32, tag=f"lk0{mtag}", name="t0")
                nc.vector.tensor_add(t0[:], ps_ap, bias_ap)
                t1 = wk.tile([128, 1], f32, tag=f"lk1{mtag}", name="t1")
                nc.vector.tensor_scalar_mul(t1[:], t0[:], 0.1)
                nc.vector.tensor_max(dst_ap, t0[:], t1[:])

            # sum partials + bias + leaky
            y1c = wk.tile([128, 2], f32, tag="y1c")
            for m in range(2):
                ps = p3.tile([128, 1], f32, tag="ps_sm")
                nc.tensor.matmul(ps[:], y1g[:, m * 128:(m + 1) * 128],
                                 ones8[:], start=True, stop=True)
                leaky(y1c[:, m:m + 1], ps[:],
                      cc[:, C32_BD1 + m:C32_BD1 + m + 1], f"y1{m}")

            def dense(y_in, wcol, bcol, oname):
                y_out = wk.tile([128, 2], f32, tag=oname)
                for m in range(2):
                    ps = p3.tile([128, 1], f32, tag="ps_sm")
                    for k in range(2):
                        nc.tensor.matmul(
                            ps[:],
                            cc[:, wcol + k * 256 + m * 128:
                                wcol + k * 256 + (m + 1) * 128],
                            y_in[:, k:k + 1], start=(k == 0), stop=(k == 1))
                    leaky(y_out[:, m:m + 1], ps[:],
                          cc[:, bcol + m:bcol + m + 1], f"{oname}{m}")
                return y_out

            y2c = dense(y1c, C32_WD2, C32_BD2, "y2c")
            y3c = dense(y2c, C32_WD3, C32_BD3, "y3c")

            ps_o = p3.tile([1, 1], f32, tag="ps_sm")
            for k in range(2):
                nc.tensor.matmul(ps_o[:],
                                 cc[:, C32_WD4 + k:C32_WD4 + k + 1],
                                 y3c[:, k:k + 1], start=(k == 0), stop=(k == 1))
            out_sb = wk.tile([1, 1], f32, tag="out_sb")
            nc.vector.tensor_add(out_sb[:], ps_o[:], cc[0:1, C32_BD4:C32_BD4 + 1])
            nc.sync.dma_start(out[:], out_sb[:])

    nc.compile()
    return nc


def _get_nc():
    if "full" not in _NC:
        _NC["full"] = _build_nc()
    return _NC["full"]


def make_in_maps(inputs):
    """Host-side sharding / preprocessing. Returns per-core input dicts."""
    x = np.asarray(inputs["x"], dtype=np.float32)
    ei = np.asarray(inputs["edge_index"])
    W1 = np.asarray(inputs["W1"], np.float32)
    W2 = np.asarray(inputs["W2"], np.float32)
    W3 = np.asarray(inputs["W3"], np.float32)
    b1 = np.asarray(inputs["b1"], np.float32)
    b2 = np.asarray(inputs["b2"], np.float32)
    b3 = np.asarray(inputs["b3"], np.float32)
    Wd1 = np.asarray(inputs["Wd1"], np.float32)
    Wd2 = np.asarray(inputs["Wd2"], np.float32)
    Wd3 = np.asarray(inputs["Wd3"], np.float32)
    Wd4 = np.asarray(inputs["Wd4"], np.float32)
    bd1 = np.asarray(inputs["bd1"], np.float32)
    bd2 = np.asarray(inputs["bd2"], np.float32)
    bd3 = np.asarray(inputs["bd3"], np.float32)
    bd4 = np.asarray(inputs["bd4"], np.float32)

    # normalized adjacency with self loops (GCNConv): A[dst, src]
    src = ei[0].astype(np.int64)
    dst = ei[1].astype(np.int64)
    loop = np.arange(N, dtype=np.int64)
    s_all = np.concatenate([src, loop])
    d_all = np.concatenate([dst, loop])
    deg = np.bincount(d_all, minlength=N).astype(np.float32)
    dinv = np.where(deg > 0, 1.0 / np.sqrt(deg), 0.0).astype(np.float32)
    wnorm = dinv[s_all] * dinv[d_all]
    A = np.zeros((N, N), np.float32)
    np.add.at(A, (d_all, s_all), wnorm)

    # atsT[p, k*1024 + d] = A[d, k*128 + p]  (A^T in [src_p, src_blk, dst])
    AT = np.zeros((P, P), np.float32)
    AT[:N, :N] = A.T
    atsT = AT.reshape(8, 128, P).transpose(1, 0, 2).reshape(128, 8 * P)

    xkp = np.zeros((P, F), np.float32)
    xkp[:N] = x
    xk = xkp.reshape(8, 128, F).transpose(1, 0, 2).reshape(128, 8 * F)

    c16a = np.zeros((128, C16A_W), np.float16)
    c16a[:, C16A_ATST:C16A_ATST + 8 * P] = atsT.astype(np.float16)
    c16a[:, C16A_XK:C16A_XK + 8 * F] = xk.astype(np.float16)
    c16a[:, C16A_W1:C16A_W1 + D] = W1.astype(np.float16)

    # w2/w3 as [p, m*512 + d] = W[m*128+p, d]
    w2l = W2.reshape(4, 128, D).transpose(1, 0, 2).reshape(128, 4 * D)
    w3l = W3.reshape(4, 128, D).transpose(1, 0, 2).reshape(128, 4 * D)

    c32 = np.zeros((128, C32_W), np.float32)
    c32[:, C32_WD2:C32_WD2 + 512] = Wd2.reshape(2, 128, H).transpose(
        1, 0, 2).reshape(128, 512)
    c32[:, C32_WD3:C32_WD3 + 512] = Wd3.reshape(2, 128, H).transpose(
        1, 0, 2).reshape(128, 512)
    c32[:, C32_WD4:C32_WD4 + 2] = Wd4.reshape(2, 128).T
    c32[:, C32_BD1:C32_BD1 + 2] = bd1.reshape(2, 128).T
    c32[:, C32_BD2:C32_BD2 + 2] = bd2.reshape(2, 128).T
    c32[:, C32_BD3:C32_BD3 + 2] = bd3.reshape(2, 128).T
    c32[0, C32_BD4] = bd4[0]
    c32[:, C32_B3C:C32_B3C + 4] = b3.reshape(4, 128).T
    c32[:, C32_B1BC:C32_B1BC + D] = np.broadcast_to(b1[None, :], (128, D))
    c32[:, C32_B2BC:C32_B2BC + D] = np.broadcast_to(b2[None, :], (128, D))

    in_maps = []
    for r in range(N_CORES):
        c16b = np.zeros((128, C16B_W), np.float16)
        c16b[:, C16B_W2:C16B_W2 + 4 * D] = w2l.astype(np.float16)
        c16b[:, C16B_W3:C16B_W3 + 4 * D] = w3l.astype(np.float16)
        # atsL[p, k*128 + dd] = A[r*125 + dd, k*128 + p]
        atsL = np.zeros((128, 8 * 128), np.float16)
        loc = AT[:, r * NL:(r + 1) * NL].reshape(8, 128, NL).transpose(
            1, 0, 2)  # [p, k, dd]
        atsLf = np.zeros((128, 8, 128), np.float32)
        atsLf[:, :, :NL] = loc
        atsL[:] = atsLf.reshape(128, 8 * 128).astype(np.float16)
        c16b[:, C16B_ATSL:C16B_ATSL + 8 * 128] = atsL

        sl = Wd1[r * NL * D:(r + 1) * NL * D]  # [64000, 256]
        # j-major, 126-chunk-padded: block (j, i) at chunk j*126+i
        blk = sl.reshape(NL, 4, 128, H).transpose(1, 0, 2, 3)  # [j, i, p, n]
        blkp = np.zeros((4, JCH, 128, H), np.float32)
        blkp[:, :NL] = blk
        wd1 = np.ascontiguousarray(
            blkp.transpose(2, 0, 1, 3).reshape(128, KCHP * H)).astype(
                np.float16)
        in_maps.append({"c16a": c16a, "c16b": c16b, "c32": c32,
                        "wd1s": wd1})
    return in_maps


def kernel(**inputs):
    from concourse.bass_utils import run_bass_kernel_spmd
    in_maps = make_in_maps(inputs)
    nc = _get_nc()
    res = run_bass_kernel_spmd(nc, in_maps, core_ids=list(range(N_CORES)))
    return np.asarray(res.results[0]["out"], np.float32).reshape(1)


# revision 10
# speedup vs baseline: 1.8561x; 1.1166x over previous
"""DeepHamCritic (3x GCNConv + dense head) on 8 trn2 NeuronCores.

v2 strategy (collective-minimal):
  - GCN layers 1+2 computed REPLICATED on every core (all 1024 padded
    nodes, fp16 matmuls vs the dense normalized adjacency) -- this
    removes the two inter-layer AllGathers entirely (each measured
    ~45us on this fabric vs ~25us of extra PE work).
  - Layer 3 computed only for the core's local 125 destination nodes,
    directly in TRANSPOSED form (h3T[j] = [feat 128, node 125]) so the
    dense head needs no PE transpose step.
  - Dense head: Wd1 [512000,256] row-sharded by node (125 nodes = 64000
    rows/core, fp16), streamed through SBUF slabs on a single DMA queue
    (one queue already saturates ~390GB/s; more queues don't add BW)
    and consumed by a PE matvec accumulated in PSUM. Slab DMAs are
    issued at program start so they overlap the whole GCN phase.
  - One tiny AllGather of the [1,256] partials at the end (the only
    collective), then the small Wd2/Wd3/Wd4 layers replicated.
"""

import numpy as np

N_CORES = 8
N = 1000          # real nodes
P = 1024          # padded nodes for GCN grid
NL = 125          # real nodes per core (head shard)
F = 128           # input features
D = 512           # GCN hidden
H = 256           # dense hidden
KCH = NL * 4      # 500 real f-chunks of 128 per core
JCH = 126         # chunks per j-block (125 real + 1 zero pad)
KCHP = JCH * 4    # 504 padded chunks, j-major layout
NPAIR = KCHP // 2  # 252 paired matmuls
SLAB_CH = 32      # chunks per DMA slab
N_SLAB = (KCHP + SLAB_CH - 1) // SLAB_CH  # 16 (last partial)
SLAB_W = SLAB_CH * H                      # 8192 fp16 cols per slab
WSLAB_BUFS = 6

# c16a columns: atsT | xk | w1
C16A_ATST = 0
C16A_XK = 8192
C16A_W1 = 9216
C16A_W = 9728
# c16b columns: w2 | w3 | atsL
C16B_W2 = 0
C16B_W3 = 2048
C16B_ATSL = 4096
C16B_W = 5120
# c32 columns: wd2 | wd3 | wd4 | bd1 | bd2 | bd3 | bd4 | b3col
C32_WD2 = 0
C32_WD3 = 512
C32_WD4 = 1024
C32_BD1 = 1026
C32_BD2 = 1028
C32_BD3 = 1030
C32_BD4 = 1032
C32_B3C = 1033
C32_B1BC = 1037
C32_B2BC = 1549
C32_W = 2061

_NC = {}


def _build_nc(reps=1, mode="full"):
    import concourse.bacc as bacc
    import concourse.mybir as mybir
    import concourse.tile as tile

    f32 = mybir.dt.float32
    f16 = mybir.dt.float16
    RG = [list(range(N_CORES))]

    nc = bacc.Bacc("TRN2", target_bir_lowering=False, debug=False,
                   num_devices=N_CORES)

    c16a = nc.dram_tensor("c16a", [128, C16A_W], f16, kind="ExternalInput")
    c16b = nc.dram_tensor("c16b", [128, C16B_W], f16, kind="ExternalInput")
    c32 = nc.dram_tensor("c32", [128, C32_W], f32, kind="ExternalInput")
    wd1s = nc.dram_tensor("wd1s", [128, KCHP * H], f16, kind="ExternalInput")
    out = nc.dram_tensor("out", [1, 1], f32, kind="ExternalOutput")

    Tanh = mybir.ActivationFunctionType.Tanh
    Lrelu = mybir.ActivationFunctionType.Lrelu
    Bypass = mybir.AluOpType.bypass

    do_gcn = mode in ("full", "gcn")
    do_head_pe = mode in ("full", "gcn_head", "head_pe")
    do_slab_dma = mode in ("full", "head_dma")
    do_tail = mode == "full"

    with tile.TileContext(nc) as tc:
        with (
            tc.tile_pool(name="wslab", bufs=WSLAB_BUFS) as wpool,
            tc.tile_pool(name="const", bufs=1) as cp,
            tc.tile_pool(name="hbuf", bufs=1) as hp,
            tc.tile_pool(name="work", bufs=2) as wk,
            tc.tile_pool(name="psum", bufs=2, space="PSUM") as pp,
            tc.tile_pool(name="ps3", bufs=2, space="PSUM") as p3,
            tc.tile_pool(name="psacc", bufs=1, space="PSUM") as pacc,
            tc.tile_pool(name="dram", bufs=1, space="DRAM") as dp,
        ):
          static_slabs = None
          if mode == "head_pe":
            static_slabs = []
            for sidx in range(4):
                st = cp.tile([128, SLAB_W], f16, tag=f"sslab{sidx}")
                nc.vector.memset(st[:], 0.001)
                static_slabs.append(st)
          for _rep in range(reps):
            # ---- all big DMAs on the sync queue, consts first ----
            ca = cp.tile([128, C16A_W], f16, tag="c16a")
            cb = cp.tile([128, C16B_W], f16, tag="c16b")
            cc = cp.tile([128, C32_W], f32, tag="c32")
            if do_gcn or do_tail:
                nc.sync.dma_start(ca[:], c16a[:])
                nc.sync.dma_start(cc[:], c32[:])
                nc.sync.dma_start(cb[:], c16b[:])
            slabs = []
            if do_slab_dma:
                for g in range(N_SLAB):
                    w = min(SLAB_W, KCHP * H - g * SLAB_W)
                    t = wpool.tile([128, SLAB_W], f16, tag="slab")
                    nc.sync.dma_start(t[:, :w],
                                      wd1s[:, g * SLAB_W:g * SLAB_W + w])
                    slabs.append(t)
            elif do_head_pe and mode == "head_pe":
                slabs = [static_slabs[g % 4] for g in range(N_SLAB)]

            ones8 = cp.tile([8, 1], f32, tag="ones8")
            nc.vector.memset(ones8[:], 1.0)

            h3T = None
            if do_gcn:
                # ============ GCN layer 1 (replicated, agg-first) ====
                # aggT [f 128, dst 1024] = sum_k xk_k^T @ atsT_k
                ps_h1 = [pp.tile([128, 512], f32, tag="ps_ag", name=f"ps_h1_{hh}")
                         for hh in range(2)]
                for k in range(8):
                    for hh in range(2):
                        nc.tensor.matmul(
                            ps_h1[hh][:],
                            ca[:, C16A_XK + k * 128:C16A_XK + (k + 1) * 128],
                            ca[:, k * 1024 + hh * 512:k * 1024 + (hh + 1) * 512],
                            start=(k == 0), stop=(k == 7))
                agg1T = []
                for hh in range(2):
                    t = hp.tile([128, 512], f16, tag=f"agg1T{hh}",
                                name=f"agg1T{hh}")
                    nc.vector.tensor_copy(t[:], ps_h1[hh][:])
                    agg1T.append(t)
                # h1[n,512] = tanh(agg1[n,:] @ W1 + b1), n-block at a time
                h1 = []
                for n in range(8):
                    ps = pp.tile([128, 512], f32, tag="ps_tr")
                    nc.tensor.matmul(ps[:],
                                     agg1T[n // 4][:, (n % 4) * 128:(n % 4 + 1) * 128],
                                     ca[:, C16A_W1:C16A_W1 + 512],
                                     start=True, stop=True)
                    hb = wk.tile([128, 512], f32, tag="hb1")
                    nc.vector.tensor_add(hb[:], ps[:],
                                         cc[:, C32_B1BC:C32_B1BC + 512])
                    t = hp.tile([128, 512], f16, tag=f"h1_{n}", name=f"h1_{n}")
                    nc.scalar.activation(t[:], hb[:], Tanh)
                    h1.append(t)

                # ============ GCN layer 2 (replicated) ===============
                agg2T = []
                for m in range(4):
                    psm = [pp.tile([128, 512], f32, tag="ps_ag", name=f"psm{hh}")
                           for hh in range(2)]
                    for k in range(8):
                        for hh in range(2):
                            nc.tensor.matmul(
                                psm[hh][:],
                                h1[k][:, m * 128:(m + 1) * 128],
                                ca[:, k * 1024 + hh * 512:k * 1024 + (hh + 1) * 512],
                                start=(k == 0), stop=(k == 7))
                    t = hp.tile([128, 1024], f16, tag=f"agg2T{m}",
                                name=f"agg2T{m}")
                    for hh in range(2):
                        nc.vector.tensor_copy(t[:, hh * 512:(hh + 1) * 512],
                                              psm[hh][:])
                    agg2T.append(t)
                h2 = []
                for n in range(8):
                    ps = pp.tile([128, 512], f32, tag="ps_tr")
                    for m in range(4):
                        nc.tensor.matmul(
                            ps[:],
                            agg2T[m][:, n * 128:(n + 1) * 128],
                            cb[:, C16B_W2 + m * 512:C16B_W2 + (m + 1) * 512],
                            start=(m == 0), stop=(m == 3))
                    hb = wk.tile([128, 512], f32, tag="hb2")
                    nc.vector.tensor_add(hb[:], ps[:],
                                         cc[:, C32_B2BC:C32_B2BC + 512])
                    t = hp.tile([128, 512], f16, tag=f"h2_{n}", name=f"h2_{n}")
                    nc.scalar.activation(t[:], hb[:], Tanh)
                    h2.append(t)

                # ======= GCN layer 3 (local 125 dst, transposed out) =
                a3T = hp.tile([128, 4 * 128], f16, tag="a3T")
                for m in range(4):
                    ps = p3.tile([128, 128], f32, tag="ps_sm")
                    for k in range(8):
                        nc.tensor.matmul(
                            ps[:],
                            h2[k][:, m * 128:(m + 1) * 128],
                            cb[:, C16B_ATSL + k * 128:C16B_ATSL + (k + 1) * 128],
                            start=(k == 0), stop=(k == 7))
                    nc.vector.tensor_copy(a3T[:, m * 128:(m + 1) * 128], ps[:])
                # h3T[j] [d 128, n 128] = tanh(sum_m w3(m,j)^T @ a3T_m + b3)
                h3T = []
                for j in range(4):
                    ps = p3.tile([128, 128], f32, tag="ps_sm")
                    for m in range(4):
                        nc.tensor.matmul(
                            ps[:],
                            cb[:, C16B_W3 + m * 512 + j * 128:
                                C16B_W3 + m * 512 + (j + 1) * 128],
                            a3T[:, m * 128:(m + 1) * 128],
                            start=(m == 0), stop=(m == 3))
                    t = wk.tile([128, 128], f16, tag=f"h3T{j}")
                    nc.scalar.activation(t[:], ps[:], Tanh,
                                         bias=cc[:, C32_B3C + j:C32_B3C + j + 1])
                    h3T.append(t)
            elif do_head_pe:
                h3T = []
                for j in range(4):
                    t = wk.tile([128, 128], f16, tag=f"h3T{j}")
                    nc.vector.memset(t[:], 0.001)
                    h3T.append(t)

            if mode == "gcn":
                out_sb = wk.tile([1, 1], f32, tag="out_sb")
                nc.vector.tensor_copy(out_sb[:], h3T[0][:1, :1])
                nc.sync.dma_start(out[:], out_sb[:])
                continue
            if mode == "head_dma":
                out_sb = wk.tile([1, 1], f32, tag="out_sb")
                nc.vector.tensor_copy(out_sb[:], slabs[-1][:1, :1])
                nc.sync.dma_start(out[:], out_sb[:])
                continue

            # ============ dense head matvec (paired chunks) ============
            ps_y = [pacc.tile([2, 512], f32, tag=f"ps_y{b}", name=f"ps_y{b}")
                    for b in range(2)]
            for g in range(N_SLAB):
                slab = slabs[g]
                npr = min(SLAB_CH // 2, NPAIR - g * (SLAB_CH // 2))
                for t2 in range(npr):
                    p = g * (SLAB_CH // 2) + t2
                    j, ip = p // 63, p % 63
                    b = p % 2
                    nc.tensor.matmul(
                        ps_y[b][:], h3T[j][:, 2 * ip:2 * ip + 2],
                        slab[:, t2 * 512:(t2 + 1) * 512],
                        start=(p < 2), stop=(p >= NPAIR - 2))
            # row 1 of each accumulator holds the other diag block; move
            # it to partition 0 via a [0,1]-selection matmul (partition-
            # base rule forbids direct partition-1 reads).
            e1 = cp.tile([2, 1], f32, tag="e1")
            nc.vector.memset(e1[:], 1.0)
            nc.vector.memset(e1[0:1, :], 0.0)
            ysb = []
            sel = []
            for b in range(2):
                t = wk.tile([2, 2 * H], f32, tag=f"ysb{b}", name=f"ysb{b}")
                nc.vector.tensor_copy(t[:], ps_y[b][:])
                ysb.append(t)
                s = p3.tile([1, 2 * H], f32, tag="ps_sm", name=f"sel{b}")
                nc.tensor.matmul(s[:], e1[:], t[:], start=True, stop=True)
                sel.append(s)
            ya = wk.tile([1, H], f32, tag="ya")
            nc.vector.tensor_add(ya[:], ysb[0][0:1, 0:H], ysb[1][0:1, 0:H])
            yb = wk.tile([1, H], f32, tag="yb")
            nc.vector.tensor_add(yb[:], ya[:], sel[0][0:1, H:2 * H])
            y1p = wk.tile([1, H], f32, tag="y1p")
            nc.vector.tensor_add(y1p[:], yb[:], sel[1][0:1, H:2 * H])

            if mode == "head_pe":
                out_sb = wk.tile([1, 1], f32, tag="out_sb")
                nc.vector.tensor_copy(out_sb[:], y1p[:1, :1])
                nc.sync.dma_start(out[:], out_sb[:])
                continue

            # ---- the only collective: gather [1,256] partials ----
            ccyi = dp.tile([1, H], f32, tag="ccyi")
            nc.sync.dma_start(ccyi[:], y1p[:])
            ccyo = dp.tile([8, H], f32, tag="ccyo", addr_space="Shared")
            nc.gpsimd.collective_compute(
                "AllGather", Bypass, replica_groups=RG,
                ins=[ccyi.opt()], outs=[ccyo.opt()])
            y1g = wk.tile([8, H], f32, tag="y1g")
            nc.sync.dma_start(y1g[:], ccyo[:])

            def leaky(dst_ap, ps_ap, bias_ap, mtag):
                t0 = wk.tile([128, 1], f32, tag=f"lk0{mtag}", name="t0")
                nc.vector.tensor_add(t0[:], ps_ap, bias_ap)
                t1 = wk.tile([128, 1], f32, tag=f"lk1{mtag}", name="t1")
                nc.vector.tensor_scalar_mul(t1[:], t0[:], 0.1)
                nc.vector.tensor_max(dst_ap, t0[:], t1[:])

            # sum partials + bias + leaky
            y1c = wk.tile([128, 2], f32, tag="y1c")
            for m in range(2):
                ps = p3.tile([128, 1], f32, tag="ps_sm")
                nc.tensor.matmul(ps[:], y1g[:, m * 128:(m + 1) * 128],
                                 ones8[:], start=True, stop=True)
                leaky(y1c[:, m:m + 1], ps[:],
                      cc[:, C32_BD1 + m:C32_BD1 + m + 1], f"y1{m}")

            def dense(y_in, wcol, bcol, oname):
                y_out = wk.tile([128, 2], f32, tag=oname)
                for m in range(2):
                    ps = p3.tile([128, 1], f32, tag="ps_sm")
                    for k in range(2):
                        nc.tensor.matmul(
                            ps[:],
                            cc[:, wcol + k * 256 + m * 128:
                                wcol + k * 256 + (m + 1) * 128],
                            y_in[:, k:k + 1], start=(k == 0), stop=(k == 1))
                    leaky(y_out[:, m:m + 1], ps[:],
                          cc[:, bcol + m:bcol + m + 1], f"{oname}{m}")
                return y_out

            y2c = dense(y1c, C32_WD2, C32_BD2, "y2c")
            y3c = dense(y2c, C32_WD3, C32_BD3, "y3c")

            ps_o = p3.tile([1, 1], f32, tag="ps_sm")
            for k in range(2):
                nc.tensor.matmul(ps_o[:],
                                 cc[:, C32_WD4 + k:C32_WD4 + k + 1],
                                 y3c[:, k:k + 1], start=(k == 0), stop=(k == 1))
            out_sb = wk.tile([1, 1], f32, tag="out_sb")
            nc.vector.tensor_add(out_sb[:], ps_o[:], cc[0:1, C32_BD4:C32_BD4 + 1])
            nc.sync.dma_start(out[:], out_sb[:])

    nc.compile()
    return nc


def _get_nc():
    if "full" not in _NC:
        _NC["full"] = _build_nc()
    return _NC["full"]


def make_in_maps(inputs):
    """Host-side sharding / preprocessing. Returns per-core input dicts."""
    x = np.asarray(inputs["x"], dtype=np.float32)
    ei = np.asarray(inputs["edge_index"])
    W1 = np.asarray(inputs["W1"], np.float32)
    W2 = np.asarray(inputs["W2"], np.float32)
    W3 = np.asarray(inputs["W3"], np.float32)
    b1 = np.asarray(inputs["b1"], np.float32)
    b2 = np.asarray(inputs["b2"], np.float32)
    b3 = np.asarray(inputs["b3"], np.float32)
    Wd1 = np.asarray(inputs["Wd1"], np.float32)
    Wd2 = np.asarray(inputs["Wd2"], np.float32)
    Wd3 = np.asarray(inputs["Wd3"], np.float32)
    Wd4 = np.asarray(inputs["Wd4"], np.float32)
    bd1 = np.asarray(inputs["bd1"], np.float32)
    bd2 = np.asarray(inputs["bd2"], np.float32)
    bd3 = np.asarray(inputs["bd3"], np.float32)
    bd4 = np.asarray(inputs["bd4"], np.float32)

    # normalized adjacency with self loops (GCNConv): A[dst, src]
    src = ei[0].astype(np.int64)
    dst = ei[1].astype(np.int64)
    loop = np.arange(N, dtype=np.int64)
    s_all = np.concatenate([src, loop])
    d_all = np.concatenate([dst, loop])
    deg = np.bincount(d_all, minlength=N).astype(np.float32)
    dinv = np.where(deg > 0, 1.0 / np.sqrt(deg), 0.0).astype(np.float32)
    wnorm = dinv[s_all] * dinv[d_all]
    A = np.zeros((N, N), np.float32)
    np.add.at(A, (d_all, s_all), wnorm)

    # atsT[p, k*1024 + d] = A[d, k*128 + p]  (A^T in [src_p, src_blk, dst])
    AT = np.zeros((P, P), np.float32)
    AT[:N, :N] = A.T
    atsT = AT.reshape(8, 128, P).transpose(1, 0, 2).reshape(128, 8 * P)

    xkp = np.zeros((P, F), np.float32)
    xkp[:N] = x
    xk = xkp.reshape(8, 128, F).transpose(1, 0, 2).reshape(128, 8 * F)

    c16a = np.zeros((128, C16A_W), np.float16)
    c16a[:, C16A_ATST:C16A_ATST + 8 * P] = atsT.astype(np.float16)
    c16a[:, C16A_XK:C16A_XK + 8 * F] = xk.astype(np.float16)
    c16a[:, C16A_W1:C16A_W1 + D] = W1.astype(np.float16)

    # w2/w3 as [p, m*512 + d] = W[m*128+p, d]
    w2l = W2.reshape(4, 128, D).transpose(1, 0, 2).reshape(128, 4 * D)
    w3l = W3.reshape(4, 128, D).transpose(1, 0, 2).reshape(128, 4 * D)

    c32 = np.zeros((128, C32_W), np.float32)
    c32[:, C32_WD2:C32_WD2 + 512] = Wd2.reshape(2, 128, H).transpose(
        1, 0, 2).reshape(128, 512)
    c32[:, C32_WD3:C32_WD3 + 512] = Wd3.reshape(2, 128, H).transpose(
        1, 0, 2).reshape(128, 512)
    c32[:, C32_WD4:C32_WD4 + 2] = Wd4.reshape(2, 128).T
    c32[:, C32_BD1:C32_BD1 + 2] = bd1.reshape(2, 128).T
    c32[:, C32_BD2:C32_BD2 + 2] = bd2.reshape(2, 128).T
    c32[:, C32_BD3:C32_BD3 + 2] = bd3.reshape(2, 128).T
    c32[0, C32_BD4] = bd4[0]
    c32[:, C32_B3C:C32_B3C + 4] = b3.reshape(4, 128).T
    c32[:, C32_B1BC:C32_B1BC + D] = np.broadcast_to(b1[None, :], (128, D))
    c32[:, C32_B2BC:C32_B2BC + D] = np.broadcast_to(b2[None, :], (128, D))

    in_maps = []
    for r in range(N_CORES):
        c16b = np.zeros((128, C16B_W), np.float16)
        c16b[:, C16B_W2:C16B_W2 + 4 * D] = w2l.astype(np.float16)
        c16b[:, C16B_W3:C16B_W3 + 4 * D] = w3l.astype(np.float16)
        # atsL[p, k*128 + dd] = A[r*125 + dd, k*128 + p]
        atsL = np.zeros((128, 8 * 128), np.float16)
        loc = AT[:, r * NL:(r + 1) * NL].reshape(8, 128, NL).transpose(
            1, 0, 2)  # [p, k, dd]
        atsLf = np.zeros((128, 8, 128), np.float32)
        atsLf[:, :, :NL] = loc
        atsL[:] = atsLf.reshape(128, 8 * 128).astype(np.float16)
        c16b[:, C16B_ATSL:C16B_ATSL + 8 * 128] = atsL

        sl = Wd1[r * NL * D:(r + 1) * NL * D]  # [64000, 256]
        # j-major, 126-chunk-padded: block (j, i) at chunk j*126+i
        blk = sl.reshape(NL, 4, 128, H).transpose(1, 0, 2, 3)  # [j, i, p, n]
        blkp = np.zeros((4, JCH, 128, H), np.float32)
        blkp[:, :NL] = blk
        wd1 = np.ascontiguousarray(
            blkp.transpose(2, 0, 1, 3).reshape(128, KCHP * H)).astype(
                np.float16)
        in_maps.append({"c16a": c16a, "c16b": c16b, "c32": c32,
                        "wd1s": wd1})
    return in_maps


def kernel(**inputs):
    from concourse.bass_utils import run_bass_kernel_spmd
    in_maps = make_in_maps(inputs)
    nc = _get_nc()
    res = run_bass_kernel_spmd(nc, in_maps, core_ids=list(range(N_CORES)))
    return np.asarray(res.results[0]["out"], np.float32).reshape(1)


# revision 11
# speedup vs baseline: 1.9561x; 1.0539x over previous
"""DeepHamCritic (3x GCNConv + dense head) on 8 trn2 NeuronCores.

v2 strategy (collective-minimal):
  - GCN layers 1+2 computed REPLICATED on every core (all 1024 padded
    nodes, fp16 matmuls vs the dense normalized adjacency) -- this
    removes the two inter-layer AllGathers entirely (each measured
    ~45us on this fabric vs ~25us of extra PE work).
  - Layer 3 computed only for the core's local 125 destination nodes,
    directly in TRANSPOSED form (h3T[j] = [feat 128, node 125]) so the
    dense head needs no PE transpose step.
  - Dense head: Wd1 [512000,256] row-sharded by node (125 nodes = 64000
    rows/core, fp16), streamed through SBUF slabs on a single DMA queue
    (one queue already saturates ~390GB/s; more queues don't add BW)
    and consumed by a PE matvec accumulated in PSUM. Slab DMAs are
    issued at program start so they overlap the whole GCN phase.
  - One tiny AllGather of the [1,256] partials at the end (the only
    collective), then the small Wd2/Wd3/Wd4 layers replicated.
"""

import numpy as np

N_CORES = 8
N = 1000          # real nodes
P = 1024          # padded nodes for GCN grid
NL = 125          # real nodes per core (head shard)
F = 128           # input features
D = 512           # GCN hidden
H = 256           # dense hidden
KCH = NL * 4      # 500 real f-chunks of 128 per core
JCH = 126         # chunks per j-block (125 real + 1 zero pad)
KCHP = JCH * 4    # 504 padded chunks, j-major layout
NPAIR = KCHP // 2  # 252 paired matmuls
SLAB_CH = 24      # chunks per DMA slab
N_SLAB = KCHP // SLAB_CH                  # 21 exact
SLAB_W = SLAB_CH * H                      # 6144 fp16 cols per slab
WSLAB_BUFS = 9

# c16a columns: atsT | xk | w1
C16A_ATST = 0
C16A_XK = 8192
C16A_W1 = 9216
C16A_W = 9728
# c16b columns: w2 | w3 | atsL
C16B_W2 = 0
C16B_W3 = 2048
C16B_ATSL = 4096
C16B_W = 5120
# c32 columns: wd2 | wd3 | wd4 | bd1 | bd2 | bd3 | bd4 | b3col
C32_WD2 = 0
C32_WD3 = 512
C32_WD4 = 1024
C32_BD1 = 1026
C32_BD2 = 1028
C32_BD3 = 1030
C32_BD4 = 1032
C32_B3C = 1033
C32_B1BC = 1037
C32_B2BC = 1549
C32_W = 2061

_NC = {}


def _build_nc(reps=1, mode="full"):
    import concourse.bacc as bacc
    import concourse.mybir as mybir
    import concourse.tile as tile

    f32 = mybir.dt.float32
    f16 = mybir.dt.float16
    RG = [list(range(N_CORES))]

    nc = bacc.Bacc("TRN2", target_bir_lowering=False, debug=False,
                   num_devices=N_CORES)

    c16a = nc.dram_tensor("c16a", [128, C16A_W], f16, kind="ExternalInput")
    c16b = nc.dram_tensor("c16b", [128, C16B_W], f16, kind="ExternalInput")
    c32 = nc.dram_tensor("c32", [128, C32_W], f32, kind="ExternalInput")
    wd1s = nc.dram_tensor("wd1s", [128, KCHP * H], f16, kind="ExternalInput")
    out = nc.dram_tensor("out", [1, 1], f32, kind="ExternalOutput")

    Tanh = mybir.ActivationFunctionType.Tanh
    Lrelu = mybir.ActivationFunctionType.Lrelu
    Bypass = mybir.AluOpType.bypass

    do_gcn = mode in ("full", "gcn")
    do_head_pe = mode in ("full", "gcn_head", "head_pe")
    do_slab_dma = mode in ("full", "head_dma")
    do_tail = mode == "full"

    with tile.TileContext(nc) as tc:
        with (
            tc.tile_pool(name="wslab", bufs=WSLAB_BUFS) as wpool,
            tc.tile_pool(name="const", bufs=1) as cp,
            tc.tile_pool(name="hbuf", bufs=1) as hp,
            tc.tile_pool(name="work", bufs=2) as wk,
            tc.tile_pool(name="psum", bufs=2, space="PSUM") as pp,
            tc.tile_pool(name="ps3", bufs=2, space="PSUM") as p3,
            tc.tile_pool(name="psacc", bufs=1, space="PSUM") as pacc,
            tc.tile_pool(name="dram", bufs=1, space="DRAM") as dp,
        ):
          static_slabs = None
          if mode == "head_pe":
            static_slabs = []
            for sidx in range(4):
                st = cp.tile([128, SLAB_W], f16, tag=f"sslab{sidx}")
                nc.vector.memset(st[:], 0.001)
                static_slabs.append(st)
          for _rep in range(reps):
            # ---- all big DMAs on the sync queue, consts first ----
            ca = cp.tile([128, C16A_W], f16, tag="c16a")
            cb = cp.tile([128, C16B_W], f16, tag="c16b")
            cc = cp.tile([128, C32_W], f32, tag="c32")
            if do_gcn or do_tail:
                nc.sync.dma_start(ca[:], c16a[:])
                nc.sync.dma_start(cc[:], c32[:])
                nc.sync.dma_start(cb[:], c16b[:])
            slabs = []
            if do_slab_dma:
                for g in range(N_SLAB):
                    w = min(SLAB_W, KCHP * H - g * SLAB_W)
                    t = wpool.tile([128, SLAB_W], f16, tag="slab")
                    nc.sync.dma_start(t[:, :w],
                                      wd1s[:, g * SLAB_W:g * SLAB_W + w])
                    slabs.append(t)
            elif do_head_pe and mode == "head_pe":
                slabs = [static_slabs[g % 4] for g in range(N_SLAB)]

            ones8 = cp.tile([8, 1], f32, tag="ones8")
            nc.vector.memset(ones8[:], 1.0)

            h3T = None
            if do_gcn:
                # ============ GCN layer 1 (replicated, agg-first) ====
                # aggT [f 128, dst 1024] = sum_k xk_k^T @ atsT_k
                ps_h1 = [pp.tile([128, 512], f32, tag="ps_ag", name=f"ps_h1_{hh}")
                         for hh in range(2)]
                for k in range(8):
                    for hh in range(2):
                        nc.tensor.matmul(
                            ps_h1[hh][:],
                            ca[:, C16A_XK + k * 128:C16A_XK + (k + 1) * 128],
                            ca[:, k * 1024 + hh * 512:k * 1024 + (hh + 1) * 512],
                            start=(k == 0), stop=(k == 7))
                agg1T = []
                for hh in range(2):
                    t = hp.tile([128, 512], f16, tag=f"agg1T{hh}",
                                name=f"agg1T{hh}")
                    nc.vector.tensor_copy(t[:], ps_h1[hh][:])
                    agg1T.append(t)
                # h1[n,512] = tanh(agg1[n,:] @ W1 + b1), n-block at a time
                h1 = []
                for n in range(8):
                    ps = pp.tile([128, 512], f32, tag="ps_tr")
                    nc.tensor.matmul(ps[:],
                                     agg1T[n // 4][:, (n % 4) * 128:(n % 4 + 1) * 128],
                                     ca[:, C16A_W1:C16A_W1 + 512],
                                     start=True, stop=True)
                    hb = wk.tile([128, 512], f32, tag="hb")
                    nc.vector.tensor_add(hb[:], ps[:],
                                         cc[:, C32_B1BC:C32_B1BC + 512])
                    t = hp.tile([128, 512], f16, tag=f"h1_{n}", name=f"h1_{n}")
                    nc.scalar.activation(t[:], hb[:], Tanh)
                    h1.append(t)

                # ============ GCN layer 2 (replicated) ===============
                agg2T = []
                for m in range(4):
                    psm = [pp.tile([128, 512], f32, tag="ps_ag", name=f"psm{hh}")
                           for hh in range(2)]
                    for k in range(8):
                        for hh in range(2):
                            nc.tensor.matmul(
                                psm[hh][:],
                                h1[k][:, m * 128:(m + 1) * 128],
                                ca[:, k * 1024 + hh * 512:k * 1024 + (hh + 1) * 512],
                                start=(k == 0), stop=(k == 7))
                    t = hp.tile([128, 1024], f16, tag=f"agg2T{m}",
                                name=f"agg2T{m}")
                    for hh in range(2):
                        nc.vector.tensor_copy(t[:, hh * 512:(hh + 1) * 512],
                                              psm[hh][:])
                    agg2T.append(t)
                h2 = []
                for n in range(8):
                    ps = pp.tile([128, 512], f32, tag="ps_tr")
                    for m in range(4):
                        nc.tensor.matmul(
                            ps[:],
                            agg2T[m][:, n * 128:(n + 1) * 128],
                            cb[:, C16B_W2 + m * 512:C16B_W2 + (m + 1) * 512],
                            start=(m == 0), stop=(m == 3))
                    hb = wk.tile([128, 512], f32, tag="hb")
                    nc.vector.tensor_add(hb[:], ps[:],
                                         cc[:, C32_B2BC:C32_B2BC + 512])
                    t = hp.tile([128, 512], f16, tag=f"h2_{n}", name=f"h2_{n}")
                    nc.scalar.activation(t[:], hb[:], Tanh)
                    h2.append(t)

                # ======= GCN layer 3 (local 125 dst, transposed out) =
                a3T = hp.tile([128, 4 * 128], f16, tag="a3T")
                for m in range(4):
                    ps = p3.tile([128, 128], f32, tag="ps_sm")
                    for k in range(8):
                        nc.tensor.matmul(
                            ps[:],
                            h2[k][:, m * 128:(m + 1) * 128],
                            cb[:, C16B_ATSL + k * 128:C16B_ATSL + (k + 1) * 128],
                            start=(k == 0), stop=(k == 7))
                    nc.vector.tensor_copy(a3T[:, m * 128:(m + 1) * 128], ps[:])
                # h3T[j] [d 128, n 128] = tanh(sum_m w3(m,j)^T @ a3T_m + b3)
                h3T = []
                for j in range(4):
                    ps = p3.tile([128, 128], f32, tag="ps_sm")
                    for m in range(4):
                        nc.tensor.matmul(
                            ps[:],
                            cb[:, C16B_W3 + m * 512 + j * 128:
                                C16B_W3 + m * 512 + (j + 1) * 128],
                            a3T[:, m * 128:(m + 1) * 128],
                            start=(m == 0), stop=(m == 3))
                    t = wk.tile([128, 128], f16, tag=f"h3T{j}")
                    nc.scalar.activation(t[:], ps[:], Tanh,
                                         bias=cc[:, C32_B3C + j:C32_B3C + j + 1])
                    h3T.append(t)
            elif do_head_pe:
                h3T = []
                for j in range(4):
                    t = wk.tile([128, 128], f16, tag=f"h3T{j}")
                    nc.vector.memset(t[:], 0.001)
                    h3T.append(t)

            if mode == "gcn":
                out_sb = wk.tile([1, 1], f32, tag="out_sb")
                nc.vector.tensor_copy(out_sb[:], h3T[0][:1, :1])
                nc.sync.dma_start(out[:], out_sb[:])
                continue
            if mode == "head_dma":
                out_sb = wk.tile([1, 1], f32, tag="out_sb")
                nc.vector.tensor_copy(out_sb[:], slabs[-1][:1, :1])
                nc.sync.dma_start(out[:], out_sb[:])
                continue

            # ============ dense head matvec (paired chunks) ============
            ps_y = [pacc.tile([2, 512], f32, tag=f"ps_y{b}", name=f"ps_y{b}")
                    for b in range(2)]
            for g in range(N_SLAB):
                slab = slabs[g]
                npr = min(SLAB_CH // 2, NPAIR - g * (SLAB_CH // 2))
                for t2 in range(npr):
                    p = g * (SLAB_CH // 2) + t2
                    j, ip = p // 63, p % 63
                    b = p % 2
                    nc.tensor.matmul(
                        ps_y[b][:], h3T[j][:, 2 * ip:2 * ip + 2],
                        slab[:, t2 * 512:(t2 + 1) * 512],
                        start=(p < 2), stop=(p >= NPAIR - 2))
            # row 1 of each accumulator holds the other diag block; move
            # it to partition 0 via a [0,1]-selection matmul (partition-
            # base rule forbids direct partition-1 reads).
            e1 = cp.tile([2, 1], f32, tag="e1")
            nc.vector.memset(e1[:], 1.0)
            nc.vector.memset(e1[0:1, :], 0.0)
            ysb = []
            sel = []
            for b in range(2):
                t = wk.tile([2, 2 * H], f32, tag="ysb", name=f"ysb{b}")
                nc.vector.tensor_copy(t[:], ps_y[b][:])
                ysb.append(t)
                s = p3.tile([1, 2 * H], f32, tag="ps_sm", name=f"sel{b}")
                nc.tensor.matmul(s[:], e1[:], t[:], start=True, stop=True)
                sel.append(s)
            ya = wk.tile([1, H], f32, tag="ya")
            nc.vector.tensor_add(ya[:], ysb[0][0:1, 0:H], ysb[1][0:1, 0:H])
            yb = wk.tile([1, H], f32, tag="yb")
            nc.vector.tensor_add(yb[:], ya[:], sel[0][0:1, H:2 * H])
            y1p = wk.tile([1, H], f32, tag="y1p")
            nc.vector.tensor_add(y1p[:], yb[:], sel[1][0:1, H:2 * H])

            if mode == "head_pe":
                out_sb = wk.tile([1, 1], f32, tag="out_sb")
                nc.vector.tensor_copy(out_sb[:], y1p[:1, :1])
                nc.sync.dma_start(out[:], out_sb[:])
                continue

            # ---- the only collective: gather [1,256] partials ----
            ccyi = dp.tile([1, H], f32, tag="ccyi")
            nc.sync.dma_start(ccyi[:], y1p[:])
            ccyo = dp.tile([8, H], f32, tag="ccyo", addr_space="Shared")
            nc.gpsimd.collective_compute(
                "AllGather", Bypass, replica_groups=RG,
                ins=[ccyi.opt()], outs=[ccyo.opt()])
            y1g = wk.tile([8, H], f32, tag="y1g")
            nc.sync.dma_start(y1g[:], ccyo[:])

            def leaky(dst_ap, ps_ap, bias_ap, mtag):
                t0 = wk.tile([128, 1], f32, tag=f"lk0{mtag}", name="t0")
                nc.vector.tensor_add(t0[:], ps_ap, bias_ap)
                t1 = wk.tile([128, 1], f32, tag=f"lk1{mtag}", name="t1")
                nc.vector.tensor_scalar_mul(t1[:], t0[:], 0.1)
                nc.vector.tensor_max(dst_ap, t0[:], t1[:])

            # sum partials + bias + leaky
            y1c = wk.tile([128, 2], f32, tag="y1c")
            for m in range(2):
                ps = p3.tile([128, 1], f32, tag="ps_sm")
                nc.tensor.matmul(ps[:], y1g[:, m * 128:(m + 1) * 128],
                                 ones8[:], start=True, stop=True)
                leaky(y1c[:, m:m + 1], ps[:],
                      cc[:, C32_BD1 + m:C32_BD1 + m + 1], f"y1{m}")

            def dense(y_in, wcol, bcol, oname):
                y_out = wk.tile([128, 2], f32, tag=oname)
                for m in range(2):
                    ps = p3.tile([128, 1], f32, tag="ps_sm")
                    for k in range(2):
                        nc.tensor.matmul(
                            ps[:],
                            cc[:, wcol + k * 256 + m * 128:
                                wcol + k * 256 + (m + 1) * 128],
                            y_in[:, k:k + 1], start=(k == 0), stop=(k == 1))
                    leaky(y_out[:, m:m + 1], ps[:],
                          cc[:, bcol + m:bcol + m + 1], f"{oname}{m}")
                return y_out

            y2c = dense(y1c, C32_WD2, C32_BD2, "y2c")
            y3c = dense(y2c, C32_WD3, C32_BD3, "y3c")

            ps_o = p3.tile([1, 1], f32, tag="ps_sm")
            for k in range(2):
                nc.tensor.matmul(ps_o[:],
                                 cc[:, C32_WD4 + k:C32_WD4 + k + 1],
                                 y3c[:, k:k + 1], start=(k == 0), stop=(k == 1))
            out_sb = wk.tile([1, 1], f32, tag="out_sb")
            nc.vector.tensor_add(out_sb[:], ps_o[:], cc[0:1, C32_BD4:C32_BD4 + 1])
            nc.sync.dma_start(out[:], out_sb[:])

    nc.compile()
    return nc


def _get_nc():
    if "full" not in _NC:
        _NC["full"] = _build_nc()
    return _NC["full"]


def make_in_maps(inputs):
    """Host-side sharding / preprocessing. Returns per-core input dicts."""
    x = np.asarray(inputs["x"], dtype=np.float32)
    ei = np.asarray(inputs["edge_index"])
    W1 = np.asarray(inputs["W1"], np.float32)
    W2 = np.asarray(inputs["W2"], np.float32)
    W3 = np.asarray(inputs["W3"], np.float32)
    b1 = np.asarray(inputs["b1"], np.float32)
    b2 = np.asarray(inputs["b2"], np.float32)
    b3 = np.asarray(inputs["b3"], np.float32)
    Wd1 = np.asarray(inputs["Wd1"], np.float32)
    Wd2 = np.asarray(inputs["Wd2"], np.float32)
    Wd3 = np.asarray(inputs["Wd3"], np.float32)
    Wd4 = np.asarray(inputs["Wd4"], np.float32)
    bd1 = np.asarray(inputs["bd1"], np.float32)
    bd2 = np.asarray(inputs["bd2"], np.float32)
    bd3 = np.asarray(inputs["bd3"], np.float32)
    bd4 = np.asarray(inputs["bd4"], np.float32)

    # normalized adjacency with self loops (GCNConv): A[dst, src]
    src = ei[0].astype(np.int64)
    dst = ei[1].astype(np.int64)
    loop = np.arange(N, dtype=np.int64)
    s_all = np.concatenate([src, loop])
    d_all = np.concatenate([dst, loop])
    deg = np.bincount(d_all, minlength=N).astype(np.float32)
    dinv = np.where(deg > 0, 1.0 / np.sqrt(deg), 0.0).astype(np.float32)
    wnorm = dinv[s_all] * dinv[d_all]
    A = np.zeros((N, N), np.float32)
    np.add.at(A, (d_all, s_all), wnorm)

    # atsT[p, k*1024 + d] = A[d, k*128 + p]  (A^T in [src_p, src_blk, dst])
    AT = np.zeros((P, P), np.float32)
    AT[:N, :N] = A.T
    atsT = AT.reshape(8, 128, P).transpose(1, 0, 2).reshape(128, 8 * P)

    xkp = np.zeros((P, F), np.float32)
    xkp[:N] = x
    xk = xkp.reshape(8, 128, F).transpose(1, 0, 2).reshape(128, 8 * F)

    c16a = np.zeros((128, C16A_W), np.float16)
    c16a[:, C16A_ATST:C16A_ATST + 8 * P] = atsT.astype(np.float16)
    c16a[:, C16A_XK:C16A_XK + 8 * F] = xk.astype(np.float16)
    c16a[:, C16A_W1:C16A_W1 + D] = W1.astype(np.float16)

    # w2/w3 as [p, m*512 + d] = W[m*128+p, d]
    w2l = W2.reshape(4, 128, D).transpose(1, 0, 2).reshape(128, 4 * D)
    w3l = W3.reshape(4, 128, D).transpose(1, 0, 2).reshape(128, 4 * D)

    c32 = np.zeros((128, C32_W), np.float32)
    c32[:, C32_WD2:C32_WD2 + 512] = Wd2.reshape(2, 128, H).transpose(
        1, 0, 2).reshape(128, 512)
    c32[:, C32_WD3:C32_WD3 + 512] = Wd3.reshape(2, 128, H).transpose(
        1, 0, 2).reshape(128, 512)
    c32[:, C32_WD4:C32_WD4 + 2] = Wd4.reshape(2, 128).T
    c32[:, C32_BD1:C32_BD1 + 2] = bd1.reshape(2, 128).T
    c32[:, C32_BD2:C32_BD2 + 2] = bd2.reshape(2, 128).T
    c32[:, C32_BD3:C32_BD3 + 2] = bd3.reshape(2, 128).T
    c32[0, C32_BD4] = bd4[0]
    c32[:, C32_B3C:C32_B3C + 4] = b3.reshape(4, 128).T
    c32[:, C32_B1BC:C32_B1BC + D] = np.broadcast_to(b1[None, :], (128, D))
    c32[:, C32_B2BC:C32_B2BC + D] = np.broadcast_to(b2[None, :], (128, D))

    in_maps = []
    for r in range(N_CORES):
        c16b = np.zeros((128, C16B_W), np.float16)
        c16b[:, C16B_W2:C16B_W2 + 4 * D] = w2l.astype(np.float16)
        c16b[:, C16B_W3:C16B_W3 + 4 * D] = w3l.astype(np.float16)
        # atsL[p, k*128 + dd] = A[r*125 + dd, k*128 + p]
        atsL = np.zeros((128, 8 * 128), np.float16)
        loc = AT[:, r * NL:(r + 1) * NL].reshape(8, 128, NL).transpose(
            1, 0, 2)  # [p, k, dd]
        atsLf = np.zeros((128, 8, 128), np.float32)
        atsLf[:, :, :NL] = loc
        atsL[:] = atsLf.reshape(128, 8 * 128).astype(np.float16)
        c16b[:, C16B_ATSL:C16B_ATSL + 8 * 128] = atsL

        sl = Wd1[r * NL * D:(r + 1) * NL * D]  # [64000, 256]
        # j-major, 126-chunk-padded: block (j, i) at chunk j*126+i
        blk = sl.reshape(NL, 4, 128, H).transpose(1, 0, 2, 3)  # [j, i, p, n]
        blkp = np.zeros((4, JCH, 128, H), np.float32)
        blkp[:, :NL] = blk
        wd1 = np.ascontiguousarray(
            blkp.transpose(2, 0, 1, 3).reshape(128, KCHP * H)).astype(
                np.float16)
        in_maps.append({"c16a": c16a, "c16b": c16b, "c32": c32,
                        "wd1s": wd1})
    return in_maps


def kernel(**inputs):
    from concourse.bass_utils import run_bass_kernel_spmd
    in_maps = make_in_maps(inputs)
    nc = _get_nc()
    res = run_bass_kernel_spmd(nc, in_maps, core_ids=list(range(N_CORES)))
    return np.asarray(res.results[0]["out"], np.float32).reshape(1)
